# revision 1
# baseline (speedup 1.0000x reference)
"""Trainium2 Bass kernel for nn_APLoss (8 NeuronCores, SPMD row-sharded).

Algorithm: every per-row quantity of the reference collapses to a 1-D
function of the row threshold th_i = f_i - 1:

  R(th)    = sum_j relu(y_j - th)^2        (over all 16384 j)
  Rpos(th) = same over the first 2048 j

Rows (positives) are sorted by f on the host (a sharding permutation --
the loss is a mean over rows, so permutation-invariant); core c gets a
contiguous band of 256 thresholds.  Each core evaluates the moment
functions M0(t) = sum relu(y-t), Q(t) = sum relu(y-t)^2 at K=32 grid
points spanning its band, then per row uses the exact 1st-order Taylor
form  R(th) = Q(t_b) - 2*eps*M0(t_b),  eps = th - t_b,  where t_b is the
greatest grid point <= th.  (The neglected terms are O(eps^2) * density,
~3e-4 relative -- far inside the 2e-2 gate.)

Device layout: partitions = (32 grid-k  x 4 j-sublane), free = j-block
(4096 = 16384/4).  Both elementwise engines produce the moments with two
passes and free-axis accumulators (no per-chunk matmuls):
  DVE:    T = max(y - t_k, 0) [accum -> M0],  ttr T*T [accum -> Q]
  ScalarE: Relu(y - t_k) [accum -> M0],  Square [accum -> Q]
A block-ones matmul folds the 4 j-sublanes, a one-hot (from an is_ge
staircase count) gathers each row's grid values in one matmul, and the
u-update / p / mean chain runs in [128,2] row layout as before.  Each
core DMAs out one scalar; the host sums the 8 partials.

Host-side prep is layout/sharding only: dtype casts, the row sort
permutation + per-core grid metadata, the gather u[index_s[rows]], and
the (k,jsub)-interleaved y broadcast.
"""

import numpy as np

try:
    import concourse.bass as bass  # noqa: F401
except ImportError:  # pragma: no cover
    import sys

    sys.path.insert(0, "/opt/trn_rl_repo")

N = 16384
P = 2048
NCORES = 8
RPC = P // NCORES  # 256 rows per core
K = 16  # grid points per core
JSUB = 8  # j sublanes per partition group (K * JSUB = 128)
FREEW = N // JSUB  # 4096 free columns
POSW = P // JSUB  # 512 pos free columns
DSPLIT = 896  # DVE covers free cols [0, DSPLIT), ScalarE the rest
GAMMA = 0.99

_NC_CACHE = {}


def _build_nc():
    import concourse.tile as tile
    from concourse import bacc, mybir

    f32 = mybir.dt.float32
    bf16 = mybir.dt.bfloat16
    Alu = mybir.AluOpType
    Act = mybir.ActivationFunctionType

    nc = bacc.Bacc("TRN2", target_bir_lowering=False, debug=False, num_devices=NCORES)

    # inputs
    yb4_d = nc.dram_tensor("yb4", [128, FREEW], bf16, kind="ExternalInput").ap()
    smf_d = nc.dram_tensor("smf", [128, 12], f32, kind="ExternalInput").ap()
    smb_d = nc.dram_tensor("smb", [128, 2], bf16, kind="ExternalInput").ap()
    row_d = nc.dram_tensor("rowb", [1, 304], bf16, kind="ExternalInput").ap()
    bo_d = nc.dram_tensor("bones", [128, K], f32, kind="ExternalInput").ap()
    eye_d = nc.dram_tensor("eye5", [5, 5], f32, kind="ExternalInput").ap()
    out = nc.dram_tensor("out", [1, 1], f32, kind="ExternalOutput").ap()

    with tile.TileContext(nc) as tc:
        with (
            tc.tile_pool(name="const", bufs=1) as cpool,
            tc.tile_pool(name="tp", bufs=4) as tpool,
            tc.tile_pool(name="small", bufs=1) as mpool,
            tc.tile_pool(name="psum", bufs=1, space="PSUM") as ppool,
        ):
            # --- small loads first (sync queue) ---
            smf = cpool.tile([128, 12], f32)
            nc.sync.dma_start(smf[:], smf_d[:])
            smb = cpool.tile([128, 2], bf16)
            nc.sync.dma_start(smb[:], smb_d[:])
            rowb = cpool.tile([1, 304], bf16)
            nc.sync.dma_start(rowb[:], row_d[:])
            eye5 = cpool.tile([5, 5], f32)
            nc.gpsimd.dma_start(eye5[:], eye_d[:])
            bones = cpool.tile([128, K], f32)
            nc.gpsimd.dma_start(bones[:], bo_d[:])

            # smf columns: 0-1 theta(f32), 2-3 ua, 4-5 up, 6 t128(+t per
            # partition group), 7 negt128, 8 t32 (rows 0-31), 9 k32 iota,
            # 10 ones128, 11 pad
            theta = smf[:, 0:2]
            ua = smf[:, 2:4]
            up = smf[:, 4:6]
            t128 = smf[:, 6:7]
            negt128 = smf[:, 7:8]
            t32 = smf[0:K, 8:9]
            k32 = smf[0:K, 9:10]
            ones128 = smf[:, 10:11]
            # smb columns: 0 tgrid bf16 (rows 0-31), 1 pad
            tgb = smb[0:K, 0:1]
            # rowb: 0-255 theta row bf16, 256-287 ones32 row, 288-303 pad
            throw = rowb[0:1, 0:256]
            ones32r = rowb[0:1, 256 : 256 + K]

            # --- big y broadcast, 4 quarters aligned to the engine split
            # (DVE reads [0:DSPLIT), ScalarE the rest).  One corner gate on
            # the pos quarter keeps the small loads ahead; the quarters are
            # independent so their descriptors generate in parallel.
            Yb = cpool.tile([128, FREEW], bf16)
            nc.gpsimd.tensor_copy(Yb[0:1, 0:1], rowb[0:1, 300:301])
            nc.gpsimd.tensor_copy(Yb[0:1, POSW : POSW + 1], rowb[0:1, 301:302])
            nc.gpsimd.tensor_copy(Yb[0:1, DSPLIT : DSPLIT + 1], rowb[0:1, 302:303])
            Q4 = 1472
            nc.gpsimd.tensor_copy(Yb[0:1, Q4 : Q4 + 1], rowb[0:1, 303:304])
            nc.gpsimd.dma_start(Yb[:, 0:POSW], yb4_d[:, 0:POSW])
            nc.gpsimd.dma_start(Yb[:, POSW:DSPLIT], yb4_d[:, POSW:DSPLIT])
            nc.sync.dma_start(Yb[:, DSPLIT:Q4], yb4_d[:, DSPLIT:Q4])
            nc.sync.dma_start(Yb[:, Q4:FREEW], yb4_d[:, Q4:FREEW])

            # early dummy activation: triggers the ACT_TABLE_LOAD while the
            # big DMAs are still in flight
            dummy = mpool.tile([128, 1], f32)
            nc.scalar.activation(dummy[:], smf[:, 11:12], Act.Relu, bias=negt128, scale=1.0)

            # u pre-scales + A grid column (early, off critical path)
            uas = mpool.tile([128, 2], f32)
            nc.vector.tensor_scalar_mul(uas[:], ua, 1.0 - GAMMA)
            ups = mpool.tile([128, 2], f32)
            nc.vector.tensor_scalar_mul(ups[:], up, 1.0 - GAMMA)
            A = mpool.tile([K, 5], bf16)
            nc.vector.tensor_copy(A[:, 4:5], tgb)

            # --- moment passes ---
            # accum column tiles: one col per piece
            accM = mpool.tile([128, 12], f32)
            accQ = mpool.tile([128, 12], f32)
            # DVE pieces over [0, DSPLIT): pos piece [0:512), rest 512-wide
            dve_bounds = [0, POSW, 512, 704, DSPLIT]
            for i in range(len(dve_bounds) - 1):
                if i == 1:
                    # --- one-hot chain, emitted ONCE after the pos piece
                    # (depends only on theta + grid; overlaps the stream) ---
                    psTB = ppool.tile([K, 256], f32)
                    nc.tensor.matmul(psTB[:], ones32r, throw, start=True, stop=True)
                    ThetaB = mpool.tile([K, 256], bf16)
                    nc.vector.tensor_copy(ThetaB[:], psTB[:])
                    selge = mpool.tile([K, 256], bf16)
                    nc.vector.tensor_scalar(selge[:], ThetaB[:], t32, 0.0, Alu.subtract, Alu.is_ge)
                    psCnt = ppool.tile([1, 256], f32)
                    nc.tensor.matmul(psCnt[:], smb[0:K, 1:2], selge[:], start=True, stop=True)
                    brow = mpool.tile([1, 256], bf16)
                    nc.vector.tensor_scalar_add(brow[:], psCnt[:], -1.0)
                    psB = ppool.tile([K, 256], f32)
                    nc.tensor.matmul(psB[:], ones32r, brow[:], start=True, stop=True)
                    Bb = mpool.tile([K, 256], bf16)
                    nc.vector.tensor_copy(Bb[:], psB[:])
                    oh = mpool.tile([K, 256], bf16)
                    nc.vector.tensor_scalar(oh[:], Bb[:], k32, 0.0, Alu.subtract, Alu.is_equal)

                lo, hi = dve_bounds[i], dve_bounds[i + 1]
                w = hi - lo
                T = tpool.tile([128, 512], bf16, tag="dt")
                nc.vector.tensor_scalar(
                    T[:, 0:w], Yb[:, lo:hi], t128, 0.0, Alu.subtract, Alu.max
                )
                T2 = tpool.tile([128, 512], bf16, tag="dq")
                nc.vector.scalar_tensor_tensor(
                    T2[:, 0:w], T[:, 0:w], 0.0, T[:, 0:w], Alu.add, Alu.mult
                )
                nc.vector.tensor_reduce(
                    accM[:, i : i + 1], T[:, 0:w], mybir.AxisListType.X, Alu.add
                )
                nc.vector.tensor_reduce(
                    accQ[:, i : i + 1], T2[:, 0:w], mybir.AxisListType.X, Alu.add
                )
            nd = len(dve_bounds) - 1
            # ScalarE pieces over [DSPLIT, FREEW)
            sc_bounds = [DSPLIT, 1280, 1664, FREEW]
            for i in range(len(sc_bounds) - 1):
                lo, hi = sc_bounds[i], sc_bounds[i + 1]
                w = hi - lo
                T = tpool.tile([128, 512], f32, tag="st")
                nc.scalar.activation(
                    T[:, 0:w], Yb[:, lo:hi], Act.Relu, bias=negt128, scale=1.0,
                    accum_out=accM[:, nd + i : nd + i + 1],
                )
                T2 = tpool.tile([128, 512], f32, tag="sq")
                nc.scalar.activation(
                    T2[:, 0:w], T[:, 0:w], Act.Square,
                    accum_out=accQ[:, nd + i : nd + i + 1],
                )
            npc = nd + len(sc_bounds) - 1  # total pieces

            # --- fold pieces: macc = {M0pos, Qpos, M0rest, Qrest} ---
            macc = mpool.tile([128, 4], f32)
            nc.vector.tensor_copy(macc[:, 0:1], accM[:, 0:1])
            nc.vector.tensor_copy(macc[:, 1:2], accQ[:, 0:1])
            nc.vector.tensor_reduce(macc[:, 2:3], accM[:, 1:npc], mybir.AxisListType.X, Alu.add)
            nc.vector.tensor_reduce(macc[:, 3:4], accQ[:, 1:npc], mybir.AxisListType.X, Alu.add)
            # fold 4 j-sublanes per grid k: psM[k] = {M0p, Qp, M0r, Qr}
            psM = ppool.tile([K, 4], f32)
            nc.tensor.matmul(psM[:], bones[:], macc[:], start=True, stop=True)
            smac = mpool.tile([K, 4], f32)
            nc.vector.tensor_copy(smac[:], psM[:])
            # A = {M0t, Qt, M0p, Qp, tgrid}
            nc.vector.tensor_tensor(A[:, 0:2], smac[:, 0:2], smac[:, 2:4], Alu.add)
            nc.vector.tensor_copy(A[:, 2:4], smac[:, 0:2])

            # --- per-row gather + epilogue ---
            ps2 = ppool.tile([5, 256], f32)
            nc.tensor.matmul(ps2[:], A[:], oh[:], start=True, stop=True)
            sb2 = mpool.tile([5, 256], f32)
            nc.vector.tensor_copy(sb2[:], ps2[:])
            psT = ppool.tile([128, 2, 5], f32)
            for h in range(2):
                nc.tensor.transpose(
                    psT[:, h, :], sb2[:, h * 128 : (h + 1) * 128], eye5[:]
                )
            V = mpool.tile([128, 2, 5], f32)
            nc.vector.tensor_copy(
                V[:].rearrange("p a b -> p (a b)"),
                psT[:].rearrange("p a b -> p (a b)"),
            )
            M0t = V[:, :, 0]
            Qt = V[:, :, 1]
            M0p = V[:, :, 2]
            Qp = V[:, :, 3]
            tb = V[:, :, 4]
            eps = mpool.tile([128, 2], f32)
            nc.vector.tensor_tensor(eps[:], theta, tb, Alu.subtract)
            e1 = mpool.tile([128, 2], f32)
            nc.vector.tensor_tensor(e1[:], eps[:], M0t, Alu.mult)
            R = mpool.tile([128, 2], f32)
            nc.vector.scalar_tensor_tensor(R[:], e1[:], -2.0, Qt, Alu.mult, Alu.add)
            e2 = mpool.tile([128, 2], f32)
            nc.vector.tensor_tensor(e2[:], eps[:], M0p, Alu.mult)
            Rp = mpool.tile([128, 2], f32)
            nc.vector.scalar_tensor_tensor(Rp[:], e2[:], -2.0, Qp, Alu.mult, Alu.add)
            uan = mpool.tile([128, 2], f32)
            nc.vector.scalar_tensor_tensor(uan[:], R[:], GAMMA / N, uas[:], Alu.mult, Alu.add)
            upn = mpool.tile([128, 2], f32)
            nc.vector.scalar_tensor_tensor(upn[:], Rp[:], GAMMA / N, ups[:], Alu.mult, Alu.add)
            inv = mpool.tile([128, 2], f32)
            nc.vector.reciprocal(inv[:], uan[:])
            g1 = mpool.tile([128, 2], f32)
            nc.vector.tensor_tensor(g1[:], upn[:], R[:], Alu.mult)
            g2 = mpool.tile([128, 2], f32)
            nc.vector.tensor_tensor(g2[:], g1[:], inv[:], Alu.mult)
            g3 = mpool.tile([128, 2], f32)
            nc.vector.tensor_tensor(g3[:], g2[:], Rp[:], Alu.subtract)
            g4 = mpool.tile([128, 2], f32)
            nc.vector.scalar_tensor_tensor(
                g4[:], g3[:], 1.0 / (float(N) * float(P)), inv[:], Alu.mult, Alu.mult
            )
            psF = ppool.tile([1, 2], f32)
            nc.tensor.matmul(psF[:], ones128, g4[:], start=True, stop=True)
            outsb = mpool.tile([1, 1], f32)
            nc.vector.tensor_reduce(outsb[:], psF[:], mybir.AxisListType.X, Alu.add)
            nc.sync.dma_start(out[:], outsb[:])

    nc.compile()
    return nc


def get_nc():
    if "nc" not in _NC_CACHE:
        _NC_CACHE["nc"] = _build_nc()
    return _NC_CACHE["nc"]


def make_in_maps(y_pred, u_all, u_pos, index_s, n_pos):
    import ml_dtypes

    y = np.ascontiguousarray(np.asarray(y_pred, dtype=np.float32).reshape(N))
    u_all = np.asarray(u_all, dtype=np.float32).reshape(-1)
    u_pos = np.asarray(u_pos, dtype=np.float32).reshape(-1)
    idx = np.asarray(index_s).astype(np.int64).reshape(-1)[:P]
    f = y[:P]

    # sharding permutation: rows sorted by f so each core gets a band
    perm = np.argsort(f, kind="stable")
    f_s = f[perm]
    ua_s = u_all[idx[perm]]
    up_s = u_pos[idx[perm]]
    th_s = f_s - 1.0

    # (k, jsub)-interleaved y broadcast: row p = 4k+jsub holds y[4j+jsub]
    y4 = y.reshape(FREEW, JSUB).T.astype(ml_dtypes.bfloat16)  # [JSUB, FREEW]
    yb4 = np.ascontiguousarray(np.tile(y4, (K, 1)))  # [128, FREEW]

    # block-ones for the jsub fold
    bones = np.zeros((128, K), dtype=np.float32)
    for k in range(K):
        bones[JSUB * k : JSUB * (k + 1), k] = 1.0

    eye5 = np.eye(5, dtype=np.float32)

    in_maps = []
    for c in range(NCORES):
        rows = slice(c * RPC, (c + 1) * RPC)
        th = th_s[rows]
        thb = th.astype(ml_dtypes.bfloat16).astype(np.float32)
        lo = float(thb.min())
        hi = float(thb.max())
        delta = max(hi - lo, 1e-3) / (K - 2)
        tg = (lo - delta / 2) + delta * np.arange(K, dtype=np.float64)
        tgb = tg.astype(np.float32).astype(ml_dtypes.bfloat16)  # snapped grid
        tgf = tgb.astype(np.float32)

        smf = np.zeros((128, 12), dtype=np.float32)
        smf[:, 0:2] = th.reshape(2, 128).T
        smf[:, 2:4] = ua_s[rows].reshape(2, 128).T
        smf[:, 4:6] = up_s[rows].reshape(2, 128).T
        smf[:, 6] = np.repeat(tgf, JSUB)  # t per partition group
        smf[:, 7] = -np.repeat(tgf, JSUB)
        smf[0:K, 8] = tgf
        smf[0:K, 9] = np.arange(K, dtype=np.float32)
        smf[:, 10] = 1.0

        smb = np.zeros((128, 2), dtype=ml_dtypes.bfloat16)
        smb[0:K, 0] = tgb
        smb[0:K, 1] = 1.0

        rowb = np.zeros((1, 304), dtype=ml_dtypes.bfloat16)
        rowb[0, 0:256] = th.astype(ml_dtypes.bfloat16)
        rowb[0, 256 : 256 + K] = 1.0

        in_maps.append(
            {
                "yb4": yb4,
                "smf": np.ascontiguousarray(smf),
                "smb": np.ascontiguousarray(smb),
                "rowb": rowb,
                "bones": bones,
                "eye5": eye5,
            }
        )
    return in_maps


def kernel(**inputs):
    n_pos = int(np.asarray(inputs["n_pos"]))
    assert n_pos == P, f"kernel hardcodes n_pos={P}, got {n_pos}"
    in_maps = make_in_maps(
        inputs["y_pred"], inputs["u_all"], inputs["u_pos"], inputs["index_s"], n_pos
    )
    from concourse.bass_utils import run_bass_kernel_spmd

    nc = get_nc()
    res = run_bass_kernel_spmd(nc, in_maps, list(range(NCORES)))
    total = 0.0
    for r in res.results:
        total += float(r["out"][0, 0])
    return np.float32(total)



# revision 13
# speedup vs baseline: 1.0907x; 1.0907x over previous
"""Trainium2 Bass kernel for nn_APLoss (8 NeuronCores, SPMD row-sharded).

Every per-row quantity of the reference collapses to a 1-D function of
the row threshold th_i = f_i - 1:

  R(th)    = sum_j relu(y_j - th)^2        (over all 16384 j)
  Rpos(th) = same over the first 2048 j

Rows (positives) are sorted by f on the host (a sharding permutation --
the loss is a mean over rows, so permutation-invariant); core c gets a
contiguous band of 256 thresholds.  Each core evaluates the moment
functions  M(t) = sum relu(y-t),  Q(t) = sum relu(y-t)^2,
C(t) = #{y > t}  at K=4 bin centers spanning its band; per row the
EXACT 2nd-order Taylor form
  R(th) = Q(t_b) - 2*eps*M(t_b) + eps^2*C(t_b),   eps = th - t_b
(nearest bin center b) leaves only the tiny crossing residual
O(rho*eps^3) -- ~2e-5 relative, far inside the 2e-2 gate.

Device layout: partitions = (4 grid-k x 32 j-sublane), free = j-block
(512 = 16384/32).  DVE computes all six moment accumulators with
accum_out / tensor_tensor_reduce (no separate reduces).  A bones
matmul folds the 32 sublanes into A3[12,2] = {M,Q,C} x {tot,pos} per
grid point; ONE matmul per 128-row half with a host-built eps-scaled
one-hot ( rows = [-2*eps*oh | oh | eps^2*oh] ) produces R and Rpos
directly in PSUM.  The u-update / p / mean chain runs in [128,2,2] row
layout; a final accum gives a [128,1] partial that each core DMAs out;
the host sums 8x128 partials.

Host-side prep is layout/sharding only: dtype casts, the row sort
permutation, per-core grid metadata (bin centers, eps, one-hot), the
gather u[index_s[rows]], and the (k,jsub)-interleaved y broadcast.
"""

import numpy as np

try:
    import concourse.bass as bass  # noqa: F401
except ImportError:  # pragma: no cover
    import sys

    sys.path.insert(0, "/opt/trn_rl_repo")

N = 16384
P = 2048
NCORES = 8
RPC = P // NCORES  # 256 rows per core
K = 4  # grid points per core
JSUB = 32  # j sublanes per partition group (K * JSUB = 128)
FREEW = N // JSUB  # 512 free columns
POSW = P // JSUB  # 64 pos free columns
GAMMA = 0.99
CP = GAMMA / ((1.0 - GAMMA) * N)  # u-update scale on R (raw-u form)
CG = 1.0 / ((1.0 - GAMMA) * float(N) * float(P))  # final scale

_NC_CACHE = {}


def _build_nc():
    import concourse.tile as tile
    from concourse import bacc, mybir

    f32 = mybir.dt.float32
    bf16 = mybir.dt.bfloat16
    Alu = mybir.AluOpType
    Act = mybir.ActivationFunctionType

    nc = bacc.Bacc("TRN2", target_bir_lowering=False, debug=False, num_devices=NCORES)

    yb_d = nc.dram_tensor("yb", [128, FREEW], bf16, kind="ExternalInput").ap()
    sm_d = nc.dram_tensor("sm", [128, 12], f32, kind="ExternalInput").ap()
    ohc_d = nc.dram_tensor("ohc", [K, 3 * 256], f32, kind="ExternalInput").ap()
    out = nc.dram_tensor("out", [128, 1], f32, kind="ExternalOutput").ap()

    with tile.TileContext(nc) as tc:
        with (
            tc.tile_pool(name="const", bufs=1) as cpool,
            tc.tile_pool(name="work", bufs=2) as wpool,
            tc.tile_pool(name="small", bufs=1) as mpool,
            tc.tile_pool(name="psum", bufs=1, space="PSUM") as ppool,
        ):
            # --- input DMAs: yb on sync, smalls on scalar (both HWDGE) ---
            Yb = cpool.tile([128, FREEW], bf16)
            nc.sync.dma_start(Yb[:], yb_d[:])
            sm = cpool.tile([128, 12], f32)
            nc.scalar.dma_start(sm[:], sm_d[:])
            ohc = cpool.tile([K, 3 * 256], f32)
            nc.scalar.dma_start(ohc[:], ohc_d[:])

            # sm columns: 0-3 U (h x {ua, up} raw), 4-7 bones4, 8 t128, 9 -t128
            bones4 = sm[:, 4:8]
            t128 = sm[:, 8:9]
            negt128 = sm[:, 9:10]

            # --- moment pass: ScalarE Relu/Square accum for full M/Q (the
            # ACT table load runs at queue-reach, before yb arrives); DVE
            # does the count pass + pos pieces with plain ops + reduces.
            # acc6 cols: [Mt, Mp, Qt, Qp, Ct, Cp]
            acc6 = mpool.tile([128, 6], f32)
            Tf = wpool.tile([128, FREEW], f32, tag="tf")
            nc.scalar.activation(
                Tf[:], Yb[:], Act.Relu, bias=negt128, scale=1.0,
                accum_out=acc6[:, 0:1],
            )
            T2f = wpool.tile([128, FREEW], f32, tag="t2f")
            nc.scalar.activation(
                T2f[:], Tf[:], Act.Square, accum_out=acc6[:, 2:3],
            )
            # count pass on DVE straight from Yb (independent of ScalarE)
            Cmf = wpool.tile([128, FREEW], bf16, tag="cmf")
            nc.vector.tensor_scalar(
                Cmf[:], Yb[:], t128, 0.0, Alu.subtract, Alu.is_gt
            )
            nc.vector.tensor_reduce(
                acc6[:, 4:5], Cmf[:], mybir.AxisListType.X, Alu.add
            )
            # pos pieces on DVE (cols [0, POSW))
            Tp = wpool.tile([128, POSW], bf16, tag="tp")
            nc.vector.tensor_scalar(
                Tp[:], Yb[:, 0:POSW], t128, 0.0, Alu.subtract, Alu.max
            )
            nc.vector.tensor_reduce(
                acc6[:, 1:2], Tp[:], mybir.AxisListType.X, Alu.add
            )
            T2p = wpool.tile([128, POSW], bf16, tag="t2p")
            nc.vector.scalar_tensor_tensor(
                T2p[:], Tp[:], 0.0, Tp[:], Alu.add, Alu.mult
            )
            nc.vector.tensor_reduce(
                acc6[:, 3:4], T2p[:], mybir.AxisListType.X, Alu.add
            )
            nc.vector.tensor_reduce(
                acc6[:, 5:6], Cmf[:, 0:POSW], mybir.AxisListType.X, Alu.add
            )

            # --- fold 32 sublanes per grid k: A6[k, (m x {tot,pos})] ---
            psA6 = ppool.tile([K, 6], f32)
            nc.tensor.matmul(psA6[:], bones4, acc6[:], start=True, stop=True)
            A6 = mpool.tile([K, 6], f32)
            nc.vector.tensor_copy(A6[:], psA6[:])

            # --- per-row gather: PSUM-accumulated eps-weighted one-hot
            # matmuls give R, Rp directly (3 m-terms per 128-row half) ---
            psRRp = ppool.tile([128, 2, 2], f32)
            for h in range(2):
                for m in range(3):
                    nc.tensor.matmul(
                        psRRp[:, h, :],
                        ohc[:, 256 * m + 128 * h : 256 * m + 128 * h + 128],
                        A6[:, 2 * m : 2 * m + 2],
                        start=(m == 0), stop=(m == 2),
                    )
            RRp = mpool.tile([128, 2, 2], f32)
            nc.vector.tensor_copy(
                RRp[:].rearrange("p a b -> p (a b)"),
                psRRp[:].rearrange("p a b -> p (a b)"),
            )

            # --- u-update / p / mean (raw-u scaling folded into CP/CG) ---
            UN = mpool.tile([128, 2, 2], f32)
            nc.vector.scalar_tensor_tensor(
                UN[:].rearrange("p a b -> p (a b)"),
                RRp[:].rearrange("p a b -> p (a b)"),
                CP,
                sm[:, 0:4],
                Alu.mult, Alu.add,
            )
            w = mpool.tile([128, 2], f32)
            nc.vector.reciprocal(w[:], UN[:, :, 0])
            t1 = mpool.tile([128, 2], f32)
            nc.vector.tensor_tensor(t1[:], UN[:, :, 1], RRp[:, :, 0], Alu.mult)
            t2 = mpool.tile([128, 2], f32)
            nc.vector.tensor_tensor(t2[:], t1[:], w[:], Alu.mult)
            t3 = mpool.tile([128, 2], f32)
            nc.vector.tensor_tensor(t3[:], t2[:], RRp[:, :, 1], Alu.subtract)
            g = mpool.tile([128, 2], f32)
            nc.vector.scalar_tensor_tensor(
                g[:], t3[:], CG, w[:], Alu.mult, Alu.mult,
            )
            gacc = mpool.tile([128, 1], f32)
            nc.vector.tensor_reduce(gacc[:], g[:], mybir.AxisListType.X, Alu.add)
            nc.sync.dma_start(out[:], gacc[:])

    nc.compile()
    return nc


def get_nc():
    if "nc" not in _NC_CACHE:
        _NC_CACHE["nc"] = _build_nc()
    return _NC_CACHE["nc"]


def make_in_maps(y_pred, u_all, u_pos, index_s, n_pos):
    import ml_dtypes

    y = np.ascontiguousarray(np.asarray(y_pred, dtype=np.float32).reshape(N))
    u_all = np.asarray(u_all, dtype=np.float32).reshape(-1)
    u_pos = np.asarray(u_pos, dtype=np.float32).reshape(-1)
    idx = np.asarray(index_s).astype(np.int64).reshape(-1)[:P]
    f = y[:P]

    # sharding permutation: rows sorted by f so each core gets a band
    perm = np.argsort(f, kind="stable")
    th_s = (f[perm] - 1.0).astype(np.float32)
    ua_s = u_all[idx[perm]]
    up_s = u_pos[idx[perm]]

    # (k, jsub)-interleaved y broadcast: row p = 32k+jsub holds y[32j+jsub]
    y4 = y.reshape(FREEW, JSUB).T.astype(ml_dtypes.bfloat16)  # [JSUB, FREEW]
    yb = np.ascontiguousarray(np.tile(y4, (K, 1)))  # [128, FREEW]

    bones4 = np.zeros((128, K), dtype=np.float32)
    for k in range(K):
        bones4[JSUB * k : JSUB * (k + 1), k] = 1.0

    in_maps = []
    for c in range(NCORES):
        rows = slice(c * RPC, (c + 1) * RPC)
        th = th_s[rows]
        lo = float(th.min())
        hi = float(th.max())
        delta = max(hi - lo, 1e-3) / K
        tg = lo + delta * (np.arange(K, dtype=np.float64) + 0.5)  # bin centers
        tgb = tg.astype(np.float32).astype(ml_dtypes.bfloat16)  # snapped
        tgf = tgb.astype(np.float32)
        b = np.clip(((th - lo) / delta).astype(np.int64), 0, K - 1)
        eps = (th - tgf[b]).astype(np.float32)

        sm = np.zeros((128, 12), dtype=np.float32)
        # U: col 2h+j = {ua, up} of row h*128 + rloc (raw, un-scaled)
        sm[:, 0] = ua_s[rows][0:128]
        sm[:, 1] = up_s[rows][0:128]
        sm[:, 2] = ua_s[rows][128:256]
        sm[:, 3] = up_s[rows][128:256]
        sm[:, 4:8] = bones4
        sm[:, 8] = np.repeat(tgf, JSUB)
        sm[:, 9] = -np.repeat(tgf, JSUB)

        # eps-scaled one-hot: [K, (m, r)] with m-blocks on the free axis
        oh = (b[None, :] == np.arange(K)[:, None]).astype(np.float32)  # [K,256]
        ohc = np.zeros((K, 3 * 256), dtype=np.float32)
        ohc[:, 0:256] = -2.0 * eps[None, :] * oh
        ohc[:, 256:512] = oh
        ohc[:, 512:768] = (eps * eps)[None, :] * oh

        in_maps.append(
            {
                "yb": yb,
                "sm": np.ascontiguousarray(sm),
                "ohc": np.ascontiguousarray(ohc),
            }
        )
    return in_maps


def kernel(**inputs):
    n_pos = int(np.asarray(inputs["n_pos"]))
    assert n_pos == P, f"kernel hardcodes n_pos={P}, got {n_pos}"
    in_maps = make_in_maps(
        inputs["y_pred"], inputs["u_all"], inputs["u_pos"], inputs["index_s"], n_pos
    )
    from concourse.bass_utils import run_bass_kernel_spmd

    nc = get_nc()
    res = run_bass_kernel_spmd(nc, in_maps, list(range(NCORES)))
    total = 0.0
    for r in res.results:
        total += float(np.asarray(r["out"], dtype=np.float64).sum())
    return np.float32(total)


# revision 15
# speedup vs baseline: 1.1065x; 1.0145x over previous
"""Trainium2 Bass kernel for nn_APLoss (8 NeuronCores, SPMD row-sharded).

Every per-row quantity of the reference collapses to a 1-D function of
the row threshold th_i = f_i - 1:

  R(th)    = sum_j relu(y_j - th)^2        (over all 16384 j)
  Rpos(th) = same over the first 2048 j

Rows (positives) are sorted by f on the host (a sharding permutation --
the loss is a mean over rows, so permutation-invariant); core c gets a
contiguous band of 256 thresholds.  Each core evaluates the moment
functions  M(t) = sum relu(y-t),  Q(t) = sum relu(y-t)^2,
C(t) = #{y > t}  at K=4 bin centers spanning its band; per row the
EXACT 2nd-order Taylor form
  R(th) = Q(t_b) - 2*eps*M(t_b) + eps^2*C(t_b),   eps = th - t_b
(nearest bin center b) leaves only the tiny crossing residual
O(rho*eps^3) -- ~2e-5 relative, far inside the 2e-2 gate.

Device layout: partitions = (4 grid-k x 32 j-sublane), free = j-block
(512 = 16384/32).  DVE computes all six moment accumulators with
accum_out / tensor_tensor_reduce (no separate reduces).  A bones
matmul folds the 32 sublanes into A3[12,2] = {M,Q,C} x {tot,pos} per
grid point; ONE matmul per 128-row half with a host-built eps-scaled
one-hot ( rows = [-2*eps*oh | oh | eps^2*oh] ) produces R and Rpos
directly in PSUM.  The u-update / p / mean chain runs in [128,2,2] row
layout; a final accum gives a [128,1] partial that each core DMAs out;
the host sums 8x128 partials.

Host-side prep is layout/sharding only: dtype casts, the row sort
permutation, per-core grid metadata (bin centers, eps, one-hot), the
gather u[index_s[rows]], and the (k,jsub)-interleaved y broadcast.
"""

import numpy as np

try:
    import concourse.bass as bass  # noqa: F401
except ImportError:  # pragma: no cover
    import sys

    sys.path.insert(0, "/opt/trn_rl_repo")

N = 16384
P = 2048
NCORES = 8
RPC = P // NCORES  # 256 rows per core
K = 4  # grid points per core
JSUB = 32  # j sublanes per partition group (K * JSUB = 128)
FREEW = N // JSUB  # 512 free columns
POSW = P // JSUB  # 64 pos free columns
GAMMA = 0.99
CP = GAMMA / ((1.0 - GAMMA) * N)  # u-update scale on R (raw-u form)
CG = 1.0 / ((1.0 - GAMMA) * float(N) * float(P))  # final scale

_NC_CACHE = {}


def _build_nc():
    import concourse.tile as tile
    from concourse import bacc, mybir

    f32 = mybir.dt.float32
    bf16 = mybir.dt.bfloat16
    Alu = mybir.AluOpType
    Act = mybir.ActivationFunctionType

    nc = bacc.Bacc("TRN2", target_bir_lowering=False, debug=False, num_devices=NCORES)

    yb_d = nc.dram_tensor("yb", [128, FREEW], bf16, kind="ExternalInput").ap()
    sm_d = nc.dram_tensor("sm", [128, 20], f32, kind="ExternalInput").ap()
    ohb_d = nc.dram_tensor("ohb", [K, 256], bf16, kind="ExternalInput").ap()
    out = nc.dram_tensor("out", [128, 1], f32, kind="ExternalOutput").ap()

    with tile.TileContext(nc) as tc:
        with (
            tc.tile_pool(name="const", bufs=1) as cpool,
            tc.tile_pool(name="work", bufs=2) as wpool,
            tc.tile_pool(name="small", bufs=1) as mpool,
            tc.tile_pool(name="psum", bufs=1, space="PSUM") as ppool,
        ):
            # --- input DMAs: yb + ohb on sync, sm on scalar (both HWDGE) ---
            Yb = cpool.tile([128, FREEW], bf16)
            nc.sync.dma_start(Yb[:], yb_d[:])
            ohb = cpool.tile([K, 256], bf16)
            nc.sync.dma_start(ohb[:], ohb_d[:])
            sm = cpool.tile([128, 20], f32)
            nc.scalar.dma_start(sm[:], sm_d[:])

            # sm columns: 0-3 U (h x {ua, up}), 4-7 bones4, 8 t128, 9 -t128,
            # 12-15 -2*eps dup, 16-19 eps^2 dup
            bones4 = sm[:, 4:8]
            t128 = sm[:, 8:9]
            negt128 = sm[:, 9:10]

            # --- moment pass: ScalarE Relu/Square accum for full M/Q (the
            # ACT table load runs at queue-reach, before yb arrives); DVE
            # does the count pass + pos pieces with plain ops + reduces.
            # acc6 cols: [Mt, Mp, Qt, Qp, Ct, Cp]
            acc6 = mpool.tile([128, 6], f32)
            Tf = wpool.tile([128, FREEW], f32, tag="tf")
            nc.scalar.activation(
                Tf[:], Yb[:], Act.Relu, bias=negt128, scale=1.0,
                accum_out=acc6[:, 0:1],
            )
            T2f = wpool.tile([128, FREEW], f32, tag="t2f")
            nc.scalar.activation(
                T2f[:], Tf[:], Act.Square, accum_out=acc6[:, 2:3],
            )
            # count pass on DVE straight from Yb (independent of ScalarE)
            Cmf = wpool.tile([128, FREEW], bf16, tag="cmf")
            nc.vector.tensor_scalar(
                Cmf[:], Yb[:], t128, 0.0, Alu.subtract, Alu.is_gt
            )
            nc.vector.tensor_reduce(
                acc6[:, 4:5], Cmf[:], mybir.AxisListType.X, Alu.add
            )
            # pos pieces on DVE (cols [0, POSW))
            Tp = wpool.tile([128, POSW], bf16, tag="tp")
            nc.vector.tensor_scalar(
                Tp[:], Yb[:, 0:POSW], t128, 0.0, Alu.subtract, Alu.max
            )
            nc.vector.tensor_reduce(
                acc6[:, 1:2], Tp[:], mybir.AxisListType.X, Alu.add
            )
            T2p = wpool.tile([128, POSW], bf16, tag="t2p")
            nc.vector.scalar_tensor_tensor(
                T2p[:], Tp[:], 0.0, Tp[:], Alu.add, Alu.mult
            )
            nc.vector.tensor_reduce(
                acc6[:, 3:4], T2p[:], mybir.AxisListType.X, Alu.add
            )
            nc.vector.tensor_reduce(
                acc6[:, 5:6], Cmf[:, 0:POSW], mybir.AxisListType.X, Alu.add
            )

            # --- fold 32 sublanes per grid k: A6[k, (m x {tot,pos})] ---
            psA6 = ppool.tile([K, 6], f32)
            nc.tensor.matmul(psA6[:], bones4, acc6[:], start=True, stop=True)
            A6b = mpool.tile([K, 6], bf16)
            nc.vector.tensor_copy(A6b[:], psA6[:])

            # --- per-row gather: one bf16 plain-one-hot matmul per half ---
            psV = ppool.tile([128, 2, 6], f32)
            for h in range(2):
                nc.tensor.matmul(
                    psV[:, h, :], ohb[:, h * 128 : (h + 1) * 128], A6b[:],
                    start=True, stop=True,
                )
            V = mpool.tile([128, 2, 6], f32)
            nc.vector.tensor_copy(
                V[:].rearrange("p a b -> p (a b)"),
                psV[:].rearrange("p a b -> p (a b)"),
            )

            # R = Q - 2*eps*M + eps^2*C on DVE (eps-dup weights from sm)
            av = mpool.tile([128, 2, 2], f32)
            nc.vector.tensor_tensor(
                av[:], sm[:, 12:16].rearrange("p (a b) -> p a b", a=2),
                V[:, :, 0:2], Alu.mult,
            )
            bv = mpool.tile([128, 2, 2], f32)
            nc.vector.tensor_tensor(
                bv[:], sm[:, 16:20].rearrange("p (a b) -> p a b", a=2),
                V[:, :, 4:6], Alu.mult,
            )
            cv = mpool.tile([128, 2, 2], f32)
            nc.vector.tensor_tensor(cv[:], av[:], bv[:], Alu.add)
            RRp = mpool.tile([128, 2, 2], f32)
            nc.vector.tensor_tensor(RRp[:], cv[:], V[:, :, 2:4], Alu.add)

            # --- u-update / p / mean (raw-u scaling folded into CP/CG) ---
            UN = mpool.tile([128, 2, 2], f32)
            nc.vector.scalar_tensor_tensor(
                UN[:].rearrange("p a b -> p (a b)"),
                RRp[:].rearrange("p a b -> p (a b)"),
                CP,
                sm[:, 0:4],
                Alu.mult, Alu.add,
            )
            w = mpool.tile([128, 2], f32)
            nc.vector.reciprocal(w[:], UN[:, :, 0])
            t1 = mpool.tile([128, 2], f32)
            nc.vector.tensor_tensor(t1[:], UN[:, :, 1], RRp[:, :, 0], Alu.mult)
            t2 = mpool.tile([128, 2], f32)
            nc.vector.tensor_tensor(t2[:], t1[:], w[:], Alu.mult)
            t3 = mpool.tile([128, 2], f32)
            nc.vector.tensor_tensor(t3[:], t2[:], RRp[:, :, 1], Alu.subtract)
            g = mpool.tile([128, 2], f32)
            nc.vector.scalar_tensor_tensor(
                g[:], t3[:], CG, w[:], Alu.mult, Alu.mult,
            )
            gacc = mpool.tile([128, 1], f32)
            nc.vector.tensor_reduce(gacc[:], g[:], mybir.AxisListType.X, Alu.add)
            nc.sync.dma_start(out[:], gacc[:])

    nc.compile()
    return nc


def get_nc():
    if "nc" not in _NC_CACHE:
        _NC_CACHE["nc"] = _build_nc()
    return _NC_CACHE["nc"]


def make_in_maps(y_pred, u_all, u_pos, index_s, n_pos):
    import ml_dtypes

    y = np.ascontiguousarray(np.asarray(y_pred, dtype=np.float32).reshape(N))
    u_all = np.asarray(u_all, dtype=np.float32).reshape(-1)
    u_pos = np.asarray(u_pos, dtype=np.float32).reshape(-1)
    idx = np.asarray(index_s).astype(np.int64).reshape(-1)[:P]
    f = y[:P]

    # sharding permutation: rows sorted by f so each core gets a band
    perm = np.argsort(f, kind="stable")
    th_s = (f[perm] - 1.0).astype(np.float32)
    ua_s = u_all[idx[perm]]
    up_s = u_pos[idx[perm]]

    # (k, jsub)-interleaved y broadcast: row p = 32k+jsub holds y[32j+jsub]
    y4 = y.reshape(FREEW, JSUB).T.astype(ml_dtypes.bfloat16)  # [JSUB, FREEW]
    yb = np.ascontiguousarray(np.tile(y4, (K, 1)))  # [128, FREEW]

    bones4 = np.zeros((128, K), dtype=np.float32)
    for k in range(K):
        bones4[JSUB * k : JSUB * (k + 1), k] = 1.0

    in_maps = []
    for c in range(NCORES):
        rows = slice(c * RPC, (c + 1) * RPC)
        th = th_s[rows]
        lo = float(th.min())
        hi = float(th.max())
        delta = max(hi - lo, 1e-3) / K
        tg = lo + delta * (np.arange(K, dtype=np.float64) + 0.5)  # bin centers
        tgb = tg.astype(np.float32).astype(ml_dtypes.bfloat16)  # snapped
        tgf = tgb.astype(np.float32)
        b = np.clip(((th - lo) / delta).astype(np.int64), 0, K - 1)
        eps = (th - tgf[b]).astype(np.float32)

        sm = np.zeros((128, 20), dtype=np.float32)
        # U: col 2h+j = {ua, up} of row h*128 + rloc (raw, un-scaled)
        sm[:, 0] = ua_s[rows][0:128]
        sm[:, 1] = up_s[rows][0:128]
        sm[:, 2] = ua_s[rows][128:256]
        sm[:, 3] = up_s[rows][128:256]
        sm[:, 4:8] = bones4
        sm[:, 8] = np.repeat(tgf, JSUB)
        sm[:, 9] = -np.repeat(tgf, JSUB)

        # eps-dup weights: col 12+2h+j = -2*eps, col 16+2h+j = eps^2
        n2e = -2.0 * eps
        ep2 = eps * eps
        for h in range(2):
            for j in range(2):
                sm[:, 12 + 2 * h + j] = n2e[128 * h : 128 * (h + 1)]
                sm[:, 16 + 2 * h + j] = ep2[128 * h : 128 * (h + 1)]

        # plain one-hot (exact in bf16)
        oh = (b[None, :] == np.arange(K)[:, None])
        ohb = np.ascontiguousarray(oh.astype(ml_dtypes.bfloat16))

        in_maps.append(
            {
                "yb": yb,
                "sm": np.ascontiguousarray(sm),
                "ohb": ohb,
            }
        )
    return in_maps


def kernel(**inputs):
    n_pos = int(np.asarray(inputs["n_pos"]))
    assert n_pos == P, f"kernel hardcodes n_pos={P}, got {n_pos}"
    in_maps = make_in_maps(
        inputs["y_pred"], inputs["u_all"], inputs["u_pos"], inputs["index_s"], n_pos
    )
    from concourse.bass_utils import run_bass_kernel_spmd

    nc = get_nc()
    res = run_bass_kernel_spmd(nc, in_maps, list(range(NCORES)))
    total = 0.0
    for r in res.results:
        total += float(np.asarray(r["out"], dtype=np.float64).sum())
    return np.float32(total)


# revision 16
# speedup vs baseline: 1.5442x; 1.3956x over previous
"""Trainium2 Bass kernel for nn_APLoss (8 NeuronCores, SPMD row-sharded).

Every per-row quantity of the reference collapses to a 1-D function of
the row threshold th_i = f_i - 1:

  R(th)    = sum_j relu(y_j - th)^2        (over all 16384 j)
  Rpos(th) = same over the first 2048 j

Rows (positives) are sorted by f on the host (a sharding permutation --
the loss is a mean over rows, so permutation-invariant); core c gets a
contiguous band of 256 thresholds.  Each core evaluates the moment
functions  M(t) = sum relu(y-t),  Q(t) = sum relu(y-t)^2,
C(t) = #{y > t}  at K=4 bin centers spanning its band; per row the
EXACT 2nd-order Taylor form
  R(th) = Q(t_b) - 2*eps*M(t_b) + eps^2*C(t_b),   eps = th - t_b
(nearest bin center b) leaves only the tiny crossing residual
O(rho*eps^3) -- ~2e-5 relative, far inside the 2e-2 gate.

Device layout: partitions = (4 grid-k x 32 j-sublane), free = j-block
(512 = 16384/32).  DVE computes all six moment accumulators with
accum_out / tensor_tensor_reduce (no separate reduces).  A bones
matmul folds the 32 sublanes into A3[12,2] = {M,Q,C} x {tot,pos} per
grid point; ONE matmul per 128-row half with a host-built eps-scaled
one-hot ( rows = [-2*eps*oh | oh | eps^2*oh] ) produces R and Rpos
directly in PSUM.  The u-update / p / mean chain runs in [128,2,2] row
layout; a final accum gives a [128,1] partial that each core DMAs out;
the host sums 8x128 partials.

Host-side prep is layout/sharding only: dtype casts, the row sort
permutation, per-core grid metadata (bin centers, eps, one-hot), the
gather u[index_s[rows]], and the (k,jsub)-interleaved y broadcast.
"""

import numpy as np

try:
    import concourse.bass as bass  # noqa: F401
except ImportError:  # pragma: no cover
    import sys

    sys.path.insert(0, "/opt/trn_rl_repo")

N = 16384
P = 2048
NCORES = 8
RPC = P // NCORES  # 256 rows per core
K = 4  # grid points per core
JSUB = 32  # j sublanes per partition group (K * JSUB = 128)
FREEW = N // JSUB  # 512 free columns
POSW = P // JSUB  # 64 pos free columns
GAMMA = 0.99
CP = GAMMA / ((1.0 - GAMMA) * N)  # u-update scale on R (raw-u form)
CG = 1.0 / ((1.0 - GAMMA) * float(N) * float(P))  # final scale

_NC_CACHE = {}


def _build_nc():
    import concourse.tile as tile
    from concourse import bacc, mybir

    f32 = mybir.dt.float32
    bf16 = mybir.dt.bfloat16
    Alu = mybir.AluOpType
    Act = mybir.ActivationFunctionType

    nc = bacc.Bacc("TRN2", target_bir_lowering=False, debug=False, num_devices=NCORES)

    yb_d = nc.dram_tensor("yb", [128, FREEW], bf16, kind="ExternalInput").ap()
    sm_d = nc.dram_tensor("sm", [128, 20], f32, kind="ExternalInput").ap()
    ohb_d = nc.dram_tensor("ohb", [K, 256], bf16, kind="ExternalInput").ap()
    out = nc.dram_tensor("out", [1, 2], f32, kind="ExternalOutput").ap()

    with tile.TileContext(nc) as tc:
        with (
            tc.tile_pool(name="const", bufs=1) as cpool,
            tc.tile_pool(name="work", bufs=2) as wpool,
            tc.tile_pool(name="small", bufs=1) as mpool,
            tc.tile_pool(name="psum", bufs=1, space="PSUM") as ppool,
        ):
            # --- input DMAs: yb + ohb on sync, sm on scalar (both HWDGE) ---
            Yb = cpool.tile([128, FREEW], bf16)
            nc.sync.dma_start(Yb[:], yb_d[:])
            ohb = cpool.tile([K, 256], bf16)
            nc.sync.dma_start(ohb[:], ohb_d[:])
            sm = cpool.tile([128, 20], f32)
            nc.scalar.dma_start(sm[:], sm_d[:])

            # sm columns: 0-3 U (h x {ua, up}), 4-7 bones4, 8 t128, 9 -t128,
            # 10 ones, 12-15 -2*eps dup, 16-19 eps^2 dup
            bones4 = sm[:, 4:8]
            t128 = sm[:, 8:9]
            negt128 = sm[:, 9:10]

            # --- moment pass: ScalarE Relu/Square accum for full M/Q (the
            # ACT table load runs at queue-reach, before yb arrives); DVE
            # does the count pass + pos pieces with plain ops + reduces.
            # acc6 cols: [Mt, Mp, Qt, Qp, Ct, Cp]
            acc6 = mpool.tile([128, 6], f32)
            Tf = wpool.tile([128, FREEW], f32, tag="tf")
            nc.scalar.activation(
                Tf[:], Yb[:], Act.Relu, bias=negt128, scale=1.0,
                accum_out=acc6[:, 0:1],
            )
            T2f = wpool.tile([128, FREEW], f32, tag="t2f")
            nc.scalar.activation(
                T2f[:], Tf[:], Act.Square, accum_out=acc6[:, 2:3],
            )
            # count pass on DVE straight from Yb (independent of ScalarE)
            Cmf = wpool.tile([128, FREEW], bf16, tag="cmf")
            nc.vector.tensor_scalar(
                Cmf[:], Yb[:], t128, 0.0, Alu.subtract, Alu.is_gt
            )
            nc.vector.tensor_reduce(
                acc6[:, 4:5], Cmf[:], mybir.AxisListType.X, Alu.add
            )
            # pos pieces on DVE (cols [0, POSW))
            Tp = wpool.tile([128, POSW], bf16, tag="tp")
            nc.vector.tensor_scalar(
                Tp[:], Yb[:, 0:POSW], t128, 0.0, Alu.subtract, Alu.max
            )
            nc.vector.tensor_reduce(
                acc6[:, 1:2], Tp[:], mybir.AxisListType.X, Alu.add
            )
            T2p = wpool.tile([128, POSW], bf16, tag="t2p")
            nc.vector.scalar_tensor_tensor(
                T2p[:], Tp[:], 0.0, Tp[:], Alu.add, Alu.mult
            )
            nc.vector.tensor_reduce(
                acc6[:, 3:4], T2p[:], mybir.AxisListType.X, Alu.add
            )
            nc.vector.tensor_reduce(
                acc6[:, 5:6], Cmf[:, 0:POSW], mybir.AxisListType.X, Alu.add
            )

            # --- fold 32 sublanes per grid k: A6[k, (m x {tot,pos})] ---
            psA6 = ppool.tile([K, 6], f32)
            nc.tensor.matmul(psA6[:], bones4, acc6[:], start=True, stop=True)
            A6b = mpool.tile([K, 6], bf16)
            nc.vector.tensor_copy(A6b[:], psA6[:])

            # --- per-row gather: one bf16 plain-one-hot matmul per half ---
            psV = ppool.tile([128, 2, 6], f32)
            for h in range(2):
                nc.tensor.matmul(
                    psV[:, h, :], ohb[:, h * 128 : (h + 1) * 128], A6b[:],
                    start=True, stop=True,
                )
            V = mpool.tile([128, 2, 6], f32)
            nc.vector.tensor_copy(
                V[:].rearrange("p a b -> p (a b)"),
                psV[:].rearrange("p a b -> p (a b)"),
            )

            # R = Q - 2*eps*M + eps^2*C on DVE (eps-dup weights from sm)
            av = mpool.tile([128, 2, 2], f32)
            nc.vector.tensor_tensor(
                av[:], sm[:, 12:16].rearrange("p (a b) -> p a b", a=2),
                V[:, :, 0:2], Alu.mult,
            )
            bv = mpool.tile([128, 2, 2], f32)
            nc.vector.tensor_tensor(
                bv[:], sm[:, 16:20].rearrange("p (a b) -> p a b", a=2),
                V[:, :, 4:6], Alu.mult,
            )
            cv = mpool.tile([128, 2, 2], f32)
            nc.vector.tensor_tensor(cv[:], av[:], bv[:], Alu.add)
            RRp = mpool.tile([128, 2, 2], f32)
            nc.vector.tensor_tensor(RRp[:], cv[:], V[:, :, 2:4], Alu.add)

            # --- u-update / p / mean (raw-u scaling folded into CP/CG) ---
            UN = mpool.tile([128, 2, 2], f32)
            nc.vector.scalar_tensor_tensor(
                UN[:].rearrange("p a b -> p (a b)"),
                RRp[:].rearrange("p a b -> p (a b)"),
                CP,
                sm[:, 0:4],
                Alu.mult, Alu.add,
            )
            w = mpool.tile([128, 2], f32)
            nc.vector.reciprocal(w[:], UN[:, :, 0])
            t1 = mpool.tile([128, 2], f32)
            nc.vector.tensor_tensor(t1[:], UN[:, :, 1], RRp[:, :, 0], Alu.mult)
            t2 = mpool.tile([128, 2], f32)
            nc.vector.tensor_tensor(t2[:], t1[:], w[:], Alu.mult)
            t3 = mpool.tile([128, 2], f32)
            nc.vector.tensor_tensor(t3[:], t2[:], RRp[:, :, 1], Alu.subtract)
            g = mpool.tile([128, 2], f32)
            nc.vector.scalar_tensor_tensor(
                g[:], t3[:], CG, w[:], Alu.mult, Alu.mult,
            )
            # fold partitions on PE so the out DMA is one 8-byte descriptor
            # (a [128,1] store pays ~6us of per-descriptor HBM write-acks)
            psF = ppool.tile([1, 2], f32)
            nc.tensor.matmul(psF[:], sm[:, 10:11], g[:], start=True, stop=True)
            outs = mpool.tile([1, 2], f32)
            nc.vector.tensor_copy(outs[:], psF[:])
            nc.sync.dma_start(out[:], outs[:])

    nc.compile()
    return nc


def get_nc():
    if "nc" not in _NC_CACHE:
        _NC_CACHE["nc"] = _build_nc()
    return _NC_CACHE["nc"]


def make_in_maps(y_pred, u_all, u_pos, index_s, n_pos):
    import ml_dtypes

    y = np.ascontiguousarray(np.asarray(y_pred, dtype=np.float32).reshape(N))
    u_all = np.asarray(u_all, dtype=np.float32).reshape(-1)
    u_pos = np.asarray(u_pos, dtype=np.float32).reshape(-1)
    idx = np.asarray(index_s).astype(np.int64).reshape(-1)[:P]
    f = y[:P]

    # sharding permutation: rows sorted by f so each core gets a band
    perm = np.argsort(f, kind="stable")
    th_s = (f[perm] - 1.0).astype(np.float32)
    ua_s = u_all[idx[perm]]
    up_s = u_pos[idx[perm]]

    # (k, jsub)-interleaved y broadcast: row p = 32k+jsub holds y[32j+jsub]
    y4 = y.reshape(FREEW, JSUB).T.astype(ml_dtypes.bfloat16)  # [JSUB, FREEW]
    yb = np.ascontiguousarray(np.tile(y4, (K, 1)))  # [128, FREEW]

    bones4 = np.zeros((128, K), dtype=np.float32)
    for k in range(K):
        bones4[JSUB * k : JSUB * (k + 1), k] = 1.0

    in_maps = []
    for c in range(NCORES):
        rows = slice(c * RPC, (c + 1) * RPC)
        th = th_s[rows]
        lo = float(th.min())
        hi = float(th.max())
        delta = max(hi - lo, 1e-3) / K
        tg = lo + delta * (np.arange(K, dtype=np.float64) + 0.5)  # bin centers
        tgb = tg.astype(np.float32).astype(ml_dtypes.bfloat16)  # snapped
        tgf = tgb.astype(np.float32)
        b = np.clip(((th - lo) / delta).astype(np.int64), 0, K - 1)
        eps = (th - tgf[b]).astype(np.float32)

        sm = np.zeros((128, 20), dtype=np.float32)
        # U: col 2h+j = {ua, up} of row h*128 + rloc (raw, un-scaled)
        sm[:, 0] = ua_s[rows][0:128]
        sm[:, 1] = up_s[rows][0:128]
        sm[:, 2] = ua_s[rows][128:256]
        sm[:, 3] = up_s[rows][128:256]
        sm[:, 4:8] = bones4
        sm[:, 8] = np.repeat(tgf, JSUB)
        sm[:, 9] = -np.repeat(tgf, JSUB)
        sm[:, 10] = 1.0

        # eps-dup weights: col 12+2h+j = -2*eps, col 16+2h+j = eps^2
        n2e = -2.0 * eps
        ep2 = eps * eps
        for h in range(2):
            for j in range(2):
                sm[:, 12 + 2 * h + j] = n2e[128 * h : 128 * (h + 1)]
                sm[:, 16 + 2 * h + j] = ep2[128 * h : 128 * (h + 1)]

        # plain one-hot (exact in bf16)
        oh = (b[None, :] == np.arange(K)[:, None])
        ohb = np.ascontiguousarray(oh.astype(ml_dtypes.bfloat16))

        in_maps.append(
            {
                "yb": yb,
                "sm": np.ascontiguousarray(sm),
                "ohb": ohb,
            }
        )
    return in_maps


def kernel(**inputs):
    n_pos = int(np.asarray(inputs["n_pos"]))
    assert n_pos == P, f"kernel hardcodes n_pos={P}, got {n_pos}"
    in_maps = make_in_maps(
        inputs["y_pred"], inputs["u_all"], inputs["u_pos"], inputs["index_s"], n_pos
    )
    from concourse.bass_utils import run_bass_kernel_spmd

    nc = get_nc()
    res = run_bass_kernel_spmd(nc, in_maps, list(range(NCORES)))
    total = 0.0
    for r in res.results:
        total += float(np.asarray(r["out"], dtype=np.float64).sum())
    return np.float32(total)


# revision 17
# speedup vs baseline: 1.5445x; 1.0002x over previous
"""Trainium2 Bass kernel for nn_APLoss (8 NeuronCores, SPMD row-sharded).

Every per-row quantity of the reference collapses to a 1-D function of
the row threshold th_i = f_i - 1:

  R(th)    = sum_j relu(y_j - th)^2        (over all 16384 j)
  Rpos(th) = same over the first 2048 j

Rows (positives) are sorted by f on the host (a sharding permutation --
the loss is a mean over rows, so permutation-invariant); core c gets a
contiguous band of 256 thresholds.  Each core evaluates the moment
functions  M(t) = sum relu(y-t),  Q(t) = sum relu(y-t)^2,
C(t) = #{y > t}  at K=4 bin centers spanning its band; per row the
EXACT 2nd-order Taylor form
  R(th) = Q(t_b) - 2*eps*M(t_b) + eps^2*C(t_b),   eps = th - t_b
(nearest bin center b) leaves only the tiny crossing residual
O(rho*eps^3) -- ~2e-5 relative, far inside the 2e-2 gate.

Device layout: partitions = (4 grid-k x 32 j-sublane), free = j-block
(512 = 16384/32).  ScalarE computes M/Q via Relu/Square activations
with free-axis accum_out (the ACT table load fires at queue-reach
time, hidden under the input DMAs); DVE computes the count pass and
the pos pieces with plain tensor_scalar + tensor_reduce.  One bones
matmul folds the 32 sublanes into A6[4,6] = k x ({M,Q,C} x {tot,pos});
one bf16 plain-one-hot matmul per 128-row half gathers each row's bin
values, and the eps-weighted Taylor combine + u-update / p / mean
chain runs on DVE in [128,2,2] row layout.  A final ones-matmul folds
the partition axis so each core stores a single [1,2] partial (a
[128,1] store costs ~6us of per-descriptor HBM write-acks); the host
sums 8 partials.

NOTE for future sessions: vector memset / iota and DVE accum_out on
tensor_scalar / scalar_tensor_tensor / tensor_tensor_reduce HANG on
this hardware path (custom-ucode ops; CoreSim and walrus accept them).
Stick to baseline-proven ops: plain TS/STT/TT, tensor_reduce,
tensor_copy, reciprocal, matmul, ScalarE activation(+accum_out).

Host-side prep is layout/sharding only: dtype casts, the row sort
permutation, per-core grid metadata (bin centers, eps, one-hot), the
gather u[index_s[rows]], and the (k,jsub)-interleaved y broadcast.
"""

import numpy as np

try:
    import concourse.bass as bass  # noqa: F401
except ImportError:  # pragma: no cover
    import sys

    sys.path.insert(0, "/opt/trn_rl_repo")

N = 16384
P = 2048
NCORES = 8
RPC = P // NCORES  # 256 rows per core
K = 4  # grid points per core
JSUB = 32  # j sublanes per partition group (K * JSUB = 128)
FREEW = N // JSUB  # 512 free columns
POSW = P // JSUB  # 64 pos free columns
GAMMA = 0.99
CP = GAMMA / ((1.0 - GAMMA) * N)  # u-update scale on R (raw-u form)
CG = 1.0 / ((1.0 - GAMMA) * float(N) * float(P))  # final scale

_NC_CACHE = {}


def _build_nc():
    import concourse.tile as tile
    from concourse import bacc, mybir

    f32 = mybir.dt.float32
    bf16 = mybir.dt.bfloat16
    Alu = mybir.AluOpType
    Act = mybir.ActivationFunctionType

    nc = bacc.Bacc("TRN2", target_bir_lowering=False, debug=False, num_devices=NCORES)

    yb_d = nc.dram_tensor("yb", [128, FREEW], bf16, kind="ExternalInput").ap()
    sm_d = nc.dram_tensor("sm", [128, 20], f32, kind="ExternalInput").ap()
    ohb_d = nc.dram_tensor("ohb", [K, 256], bf16, kind="ExternalInput").ap()
    out = nc.dram_tensor("out", [1, 2], f32, kind="ExternalOutput").ap()

    with tile.TileContext(nc) as tc:
        with (
            tc.tile_pool(name="const", bufs=1) as cpool,
            tc.tile_pool(name="work", bufs=2) as wpool,
            tc.tile_pool(name="small", bufs=1) as mpool,
            tc.tile_pool(name="psum", bufs=1, space="PSUM") as ppool,
        ):
            # --- input DMAs: yb + ohb on sync, sm on scalar (both HWDGE) ---
            Yb = cpool.tile([128, FREEW], bf16)
            nc.sync.dma_start(Yb[:], yb_d[:])
            ohb = cpool.tile([K, 256], bf16)
            nc.sync.dma_start(ohb[:], ohb_d[:])
            sm = cpool.tile([128, 20], f32)
            nc.scalar.dma_start(sm[:], sm_d[:])

            # sm columns: 0-3 U (h x {ua, up}), 4-7 bones4, 8 t128, 9 -t128,
            # 10 ones, 12-15 -2*eps dup, 16-19 eps^2 dup
            bones4 = sm[:, 4:8]
            t128 = sm[:, 8:9]
            negt128 = sm[:, 9:10]

            # --- moment pass: ScalarE Relu/Square accum for full M/Q (the
            # ACT table load runs at queue-reach, before yb arrives); DVE
            # does the count pass + pos pieces with plain ops + reduces.
            # acc6 cols: [Mt, Mp, Qt, Qp, Ct, Cp]
            acc6 = mpool.tile([128, 6], f32)
            Tf = wpool.tile([128, FREEW], f32, tag="tf")
            nc.scalar.activation(
                Tf[:], Yb[:], Act.Relu, bias=negt128, scale=1.0,
                accum_out=acc6[:, 0:1],
            )
            T2f = wpool.tile([128, FREEW], f32, tag="t2f")
            nc.scalar.activation(
                T2f[:], Tf[:], Act.Square, accum_out=acc6[:, 2:3],
            )
            # count pass on DVE straight from Yb (independent of ScalarE)
            Cmf = wpool.tile([128, FREEW], bf16, tag="cmf")
            nc.vector.tensor_scalar(
                Cmf[:], Yb[:], t128, 0.0, Alu.subtract, Alu.is_gt
            )
            nc.vector.tensor_reduce(
                acc6[:, 4:5], Cmf[:], mybir.AxisListType.X, Alu.add
            )
            # pos pieces on DVE (cols [0, POSW))
            Tp = wpool.tile([128, POSW], bf16, tag="tp")
            nc.vector.tensor_scalar(
                Tp[:], Yb[:, 0:POSW], t128, 0.0, Alu.subtract, Alu.max
            )
            nc.vector.tensor_reduce(
                acc6[:, 1:2], Tp[:], mybir.AxisListType.X, Alu.add
            )
            T2p = wpool.tile([128, POSW], bf16, tag="t2p")
            nc.vector.scalar_tensor_tensor(
                T2p[:], Tp[:], 0.0, Tp[:], Alu.add, Alu.mult
            )
            nc.vector.tensor_reduce(
                acc6[:, 3:4], T2p[:], mybir.AxisListType.X, Alu.add
            )
            nc.vector.tensor_reduce(
                acc6[:, 5:6], Cmf[:, 0:POSW], mybir.AxisListType.X, Alu.add
            )

            # --- fold 32 sublanes per grid k: A6[k, (m x {tot,pos})] ---
            psA6 = ppool.tile([K, 6], f32)
            nc.tensor.matmul(psA6[:], bones4, acc6[:], start=True, stop=True)
            A6b = mpool.tile([K, 6], bf16)
            nc.vector.tensor_copy(A6b[:], psA6[:])

            # --- per-row gather: one bf16 plain-one-hot matmul per half ---
            psV = ppool.tile([128, 2, 6], f32)
            for h in range(2):
                nc.tensor.matmul(
                    psV[:, h, :], ohb[:, h * 128 : (h + 1) * 128], A6b[:],
                    start=True, stop=True,
                )
            V = mpool.tile([128, 2, 6], f32)
            nc.vector.tensor_copy(
                V[:].rearrange("p a b -> p (a b)"),
                psV[:].rearrange("p a b -> p (a b)"),
            )

            # R = Q - 2*eps*M + eps^2*C on DVE (eps-dup weights from sm)
            av = mpool.tile([128, 2, 2], f32)
            nc.vector.tensor_tensor(
                av[:], sm[:, 12:16].rearrange("p (a b) -> p a b", a=2),
                V[:, :, 0:2], Alu.mult,
            )
            bv = mpool.tile([128, 2, 2], f32)
            nc.vector.tensor_tensor(
                bv[:], sm[:, 16:20].rearrange("p (a b) -> p a b", a=2),
                V[:, :, 4:6], Alu.mult,
            )
            cv = mpool.tile([128, 2, 2], f32)
            nc.vector.tensor_tensor(cv[:], av[:], bv[:], Alu.add)
            RRp = mpool.tile([128, 2, 2], f32)
            nc.vector.tensor_tensor(RRp[:], cv[:], V[:, :, 2:4], Alu.add)

            # --- u-update / p / mean (raw-u scaling folded into CP/CG) ---
            UN = mpool.tile([128, 2, 2], f32)
            nc.vector.scalar_tensor_tensor(
                UN[:].rearrange("p a b -> p (a b)"),
                RRp[:].rearrange("p a b -> p (a b)"),
                CP,
                sm[:, 0:4],
                Alu.mult, Alu.add,
            )
            w = mpool.tile([128, 2], f32)
            nc.vector.reciprocal(w[:], UN[:, :, 0])
            t1 = mpool.tile([128, 2], f32)
            nc.vector.tensor_tensor(t1[:], UN[:, :, 1], RRp[:, :, 0], Alu.mult)
            t2 = mpool.tile([128, 2], f32)
            nc.vector.tensor_tensor(t2[:], t1[:], w[:], Alu.mult)
            t3 = mpool.tile([128, 2], f32)
            nc.vector.tensor_tensor(t3[:], t2[:], RRp[:, :, 1], Alu.subtract)
            g = mpool.tile([128, 2], f32)
            nc.vector.scalar_tensor_tensor(
                g[:], t3[:], CG, w[:], Alu.mult, Alu.mult,
            )
            # fold partitions on PE so the out DMA is one 8-byte descriptor
            # (a [128,1] store pays ~6us of per-descriptor HBM write-acks)
            psF = ppool.tile([1, 2], f32)
            nc.tensor.matmul(psF[:], sm[:, 10:11], g[:], start=True, stop=True)
            outs = mpool.tile([1, 2], f32)
            nc.vector.tensor_copy(outs[:], psF[:])
            nc.sync.dma_start(out[:], outs[:])

    nc.compile()
    return nc


def get_nc():
    if "nc" not in _NC_CACHE:
        _NC_CACHE["nc"] = _build_nc()
    return _NC_CACHE["nc"]


def make_in_maps(y_pred, u_all, u_pos, index_s, n_pos):
    import ml_dtypes

    y = np.ascontiguousarray(np.asarray(y_pred, dtype=np.float32).reshape(N))
    u_all = np.asarray(u_all, dtype=np.float32).reshape(-1)
    u_pos = np.asarray(u_pos, dtype=np.float32).reshape(-1)
    idx = np.asarray(index_s).astype(np.int64).reshape(-1)[:P]
    f = y[:P]

    # sharding permutation: rows sorted by f so each core gets a band
    perm = np.argsort(f, kind="stable")
    th_s = (f[perm] - 1.0).astype(np.float32)
    ua_s = u_all[idx[perm]]
    up_s = u_pos[idx[perm]]

    # (k, jsub)-interleaved y broadcast: row p = 32k+jsub holds y[32j+jsub]
    y4 = y.reshape(FREEW, JSUB).T.astype(ml_dtypes.bfloat16)  # [JSUB, FREEW]
    yb = np.ascontiguousarray(np.tile(y4, (K, 1)))  # [128, FREEW]

    bones4 = np.zeros((128, K), dtype=np.float32)
    for k in range(K):
        bones4[JSUB * k : JSUB * (k + 1), k] = 1.0

    in_maps = []
    for c in range(NCORES):
        rows = slice(c * RPC, (c + 1) * RPC)
        th = th_s[rows]
        lo = float(th.min())
        hi = float(th.max())
        delta = max(hi - lo, 1e-3) / K
        tg = lo + delta * (np.arange(K, dtype=np.float64) + 0.5)  # bin centers
        tgb = tg.astype(np.float32).astype(ml_dtypes.bfloat16)  # snapped
        tgf = tgb.astype(np.float32)
        b = np.clip(((th - lo) / delta).astype(np.int64), 0, K - 1)
        eps = (th - tgf[b]).astype(np.float32)

        sm = np.zeros((128, 20), dtype=np.float32)
        # U: col 2h+j = {ua, up} of row h*128 + rloc (raw, un-scaled)
        sm[:, 0] = ua_s[rows][0:128]
        sm[:, 1] = up_s[rows][0:128]
        sm[:, 2] = ua_s[rows][128:256]
        sm[:, 3] = up_s[rows][128:256]
        sm[:, 4:8] = bones4
        sm[:, 8] = np.repeat(tgf, JSUB)
        sm[:, 9] = -np.repeat(tgf, JSUB)
        sm[:, 10] = 1.0

        # eps-dup weights: col 12+2h+j = -2*eps, col 16+2h+j = eps^2
        n2e = -2.0 * eps
        ep2 = eps * eps
        for h in range(2):
            for j in range(2):
                sm[:, 12 + 2 * h + j] = n2e[128 * h : 128 * (h + 1)]
                sm[:, 16 + 2 * h + j] = ep2[128 * h : 128 * (h + 1)]

        # plain one-hot (exact in bf16)
        oh = (b[None, :] == np.arange(K)[:, None])
        ohb = np.ascontiguousarray(oh.astype(ml_dtypes.bfloat16))

        in_maps.append(
            {
                "yb": yb,
                "sm": np.ascontiguousarray(sm),
                "ohb": ohb,
            }
        )
    return in_maps


def kernel(**inputs):
    n_pos = int(np.asarray(inputs["n_pos"]))
    assert n_pos == P, f"kernel hardcodes n_pos={P}, got {n_pos}"
    in_maps = make_in_maps(
        inputs["y_pred"], inputs["u_all"], inputs["u_pos"], inputs["index_s"], n_pos
    )
    from concourse.bass_utils import run_bass_kernel_spmd

    nc = get_nc()
    res = run_bass_kernel_spmd(nc, in_maps, list(range(NCORES)))
    total = 0.0
    for r in res.results:
        total += float(np.asarray(r["out"], dtype=np.float64).sum())
    return np.float32(total)


# revision 18
# speedup vs baseline: 1.5449x; 1.0002x over previous
"""Trainium2 Bass kernel for nn_APLoss (8 NeuronCores, SPMD row-sharded).

Every per-row quantity of the reference collapses to a 1-D function of
the row threshold th_i = f_i - 1:

  R(th)    = sum_j relu(y_j - th)^2        (over all 16384 j)
  Rpos(th) = same over the first 2048 j

Rows (positives) are sorted by f on the host (a sharding permutation --
the loss is a mean over rows, so permutation-invariant); core c gets a
contiguous band of 256 thresholds.  Each core evaluates the moment
functions  M(t) = sum relu(y-t),  Q(t) = sum relu(y-t)^2,
C(t) = #{y > t}  at K=4 bin centers spanning its band; per row the
EXACT 2nd-order Taylor form
  R(th) = Q(t_b) - 2*eps*M(t_b) + eps^2*C(t_b),   eps = th - t_b
(nearest bin center b) leaves only the tiny crossing residual
O(rho*eps^3) -- ~2e-5 relative, far inside the 2e-2 gate.

Device layout: partitions = (4 grid-k x 32 j-sublane), free = j-block
(512 = 16384/32).  ScalarE computes M/Q via Relu/Square activations
with free-axis accum_out (the ACT table load fires at queue-reach
time, hidden under the input DMAs); DVE computes the count pass and
the pos pieces with plain tensor_scalar + tensor_reduce.  One bones
matmul folds the 32 sublanes into A6[4,6] = k x ({M,Q,C} x {tot,pos});
one bf16 plain-one-hot matmul per 128-row half gathers each row's bin
values, and the eps-weighted Taylor combine + u-update / p / mean
chain runs on DVE in [128,2,2] row layout.  A final ones-matmul folds
the partition axis so each core stores a single [1,2] partial (a
[128,1] store costs ~6us of per-descriptor HBM write-acks); the host
sums 8 partials.

NOTE for future sessions: vector memset / iota and DVE accum_out on
tensor_scalar / scalar_tensor_tensor / tensor_tensor_reduce HANG on
this hardware path (custom-ucode ops; CoreSim and walrus accept them).
Stick to baseline-proven ops: plain TS/STT/TT, tensor_reduce,
tensor_copy, reciprocal, matmul, ScalarE activation(+accum_out).

Host-side prep is layout/sharding only: dtype casts, the row sort
permutation, per-core grid metadata (bin centers, eps, one-hot), the
gather u[index_s[rows]], and the (k,jsub)-interleaved y broadcast.
"""

import numpy as np

try:
    import concourse.bass as bass  # noqa: F401
except ImportError:  # pragma: no cover
    import sys

    sys.path.insert(0, "/opt/trn_rl_repo")

N = 16384
P = 2048
NCORES = 8
RPC = P // NCORES  # 256 rows per core
K = 4  # grid points per core
JSUB = 32  # j sublanes per partition group (K * JSUB = 128)
FREEW = N // JSUB  # 512 free columns
POSW = P // JSUB  # 64 pos free columns
GAMMA = 0.99
CP = GAMMA / ((1.0 - GAMMA) * N)  # u-update scale on R (raw-u form)
CG = 1.0 / ((1.0 - GAMMA) * float(N) * float(P))  # final scale

_NC_CACHE = {}


def _build_nc():
    import concourse.tile as tile
    from concourse import bacc, mybir

    f32 = mybir.dt.float32
    bf16 = mybir.dt.bfloat16
    Alu = mybir.AluOpType
    Act = mybir.ActivationFunctionType

    nc = bacc.Bacc("TRN2", target_bir_lowering=False, debug=False, num_devices=NCORES)

    yb_d = nc.dram_tensor("yb", [1, 128 * FREEW], bf16, kind="ExternalInput").ap()
    sm_d = nc.dram_tensor("sm", [1, 128 * 20], f32, kind="ExternalInput").ap()
    ohb_d = nc.dram_tensor("ohb", [K, 256], bf16, kind="ExternalInput").ap()
    out = nc.dram_tensor("out", [1, 2], f32, kind="ExternalOutput").ap()

    with tile.TileContext(nc) as tc:
        with (
            tc.tile_pool(name="const", bufs=1) as cpool,
            tc.tile_pool(name="work", bufs=2) as wpool,
            tc.tile_pool(name="small", bufs=1) as mpool,
            tc.tile_pool(name="psum", bufs=1, space="PSUM") as ppool,
        ):
            # --- input DMAs: yb + ohb on sync, sm on scalar (both HWDGE) ---
            Yb = cpool.tile([128, FREEW], bf16)
            nc.sync.dma_start(
                Yb[:], yb_d[:].rearrange("o (p f) -> (o p) f", p=128)
            )
            ohb = cpool.tile([K, 256], bf16)
            nc.sync.dma_start(ohb[:], ohb_d[:])
            sm = cpool.tile([128, 20], f32)
            nc.scalar.dma_start(
                sm[:], sm_d[:].rearrange("o (p f) -> (o p) f", p=128)
            )

            # sm columns: 0-3 U (h x {ua, up}), 4-7 bones4, 8 t128, 9 -t128,
            # 10 ones, 12-15 -2*eps dup, 16-19 eps^2 dup
            bones4 = sm[:, 4:8]
            t128 = sm[:, 8:9]
            negt128 = sm[:, 9:10]

            # --- moment pass: ScalarE Relu/Square accum for full M/Q (the
            # ACT table load runs at queue-reach, before yb arrives); DVE
            # does the count pass + pos pieces with plain ops + reduces.
            # acc6 cols: [Mt, Mp, Ct, Cp, Qt, Qp]
            acc6 = mpool.tile([128, 6], f32)
            Tf = wpool.tile([128, FREEW], f32, tag="tf")
            nc.scalar.activation(
                Tf[:], Yb[:], Act.Relu, bias=negt128, scale=1.0,
                accum_out=acc6[:, 0:1],
            )
            T2f = wpool.tile([128, FREEW], f32, tag="t2f")
            nc.scalar.activation(
                T2f[:], Tf[:], Act.Square, accum_out=acc6[:, 4:5],
            )
            # count pass on DVE straight from Yb (independent of ScalarE)
            Cmf = wpool.tile([128, FREEW], bf16, tag="cmf")
            nc.vector.tensor_scalar(
                Cmf[:], Yb[:], t128, 0.0, Alu.subtract, Alu.is_gt
            )
            nc.vector.tensor_reduce(
                acc6[:, 2:3], Cmf[:], mybir.AxisListType.X, Alu.add
            )
            # pos pieces on DVE (cols [0, POSW))
            Tp = wpool.tile([128, POSW], bf16, tag="tp")
            nc.vector.tensor_scalar(
                Tp[:], Yb[:, 0:POSW], t128, 0.0, Alu.subtract, Alu.max
            )
            nc.vector.tensor_reduce(
                acc6[:, 1:2], Tp[:], mybir.AxisListType.X, Alu.add
            )
            T2p = wpool.tile([128, POSW], bf16, tag="t2p")
            nc.vector.scalar_tensor_tensor(
                T2p[:], Tp[:], 0.0, Tp[:], Alu.add, Alu.mult
            )
            nc.vector.tensor_reduce(
                acc6[:, 5:6], T2p[:], mybir.AxisListType.X, Alu.add
            )
            nc.vector.tensor_reduce(
                acc6[:, 3:4], Cmf[:, 0:POSW], mybir.AxisListType.X, Alu.add
            )

            # --- fold 32 sublanes per grid k: A6[k, (m x {tot,pos})] ---
            psA6 = ppool.tile([K, 6], f32)
            nc.tensor.matmul(psA6[:], bones4, acc6[:], start=True, stop=True)
            A6b = mpool.tile([K, 6], bf16)
            nc.vector.tensor_copy(A6b[:], psA6[:])

            # --- per-row gather: one bf16 plain-one-hot matmul per half ---
            psV = ppool.tile([128, 2, 6], f32)
            for h in range(2):
                nc.tensor.matmul(
                    psV[:, h, :], ohb[:, h * 128 : (h + 1) * 128], A6b[:],
                    start=True, stop=True,
                )
            V = mpool.tile([128, 2, 6], f32)
            nc.vector.tensor_copy(
                V[:].rearrange("p a b -> p (a b)"),
                psV[:].rearrange("p a b -> p (a b)"),
            )

            # R = Q - 2*eps*M + eps^2*C on DVE; the {-2eps*M, eps^2*C}
            # products run as ONE op over a [128,2,2,2] AP (sm cols 12-19
            # hold the weights in matching (h, g, j) order)
            ab = mpool.tile([128, 2, 2, 2], f32)
            nc.vector.tensor_tensor(
                ab[:],
                sm[:, 12:20].rearrange("p (a g b) -> p a g b", a=2, g=2),
                V[:, :, 0:4].rearrange("p a (g b) -> p a g b", g=2),
                Alu.mult,
            )
            cv = mpool.tile([128, 2, 2], f32)
            nc.vector.tensor_tensor(cv[:], ab[:, :, 0, :], ab[:, :, 1, :], Alu.add)
            RRp = mpool.tile([128, 2, 2], f32)
            nc.vector.tensor_tensor(RRp[:], cv[:], V[:, :, 4:6], Alu.add)

            # --- u-update / p / mean (raw-u scaling folded into CP/CG) ---
            UN = mpool.tile([128, 2, 2], f32)
            nc.vector.scalar_tensor_tensor(
                UN[:].rearrange("p a b -> p (a b)"),
                RRp[:].rearrange("p a b -> p (a b)"),
                CP,
                sm[:, 0:4],
                Alu.mult, Alu.add,
            )
            w = mpool.tile([128, 2], f32)
            nc.vector.reciprocal(w[:], UN[:, :, 0])
            t1 = mpool.tile([128, 2], f32)
            nc.vector.tensor_tensor(t1[:], UN[:, :, 1], RRp[:, :, 0], Alu.mult)
            t2 = mpool.tile([128, 2], f32)
            nc.vector.tensor_tensor(t2[:], t1[:], w[:], Alu.mult)
            t3 = mpool.tile([128, 2], f32)
            nc.vector.tensor_tensor(t3[:], t2[:], RRp[:, :, 1], Alu.subtract)
            g = mpool.tile([128, 2], f32)
            nc.vector.scalar_tensor_tensor(
                g[:], t3[:], CG, w[:], Alu.mult, Alu.mult,
            )
            # fold partitions on PE so the out DMA is one 8-byte descriptor
            # (a [128,1] store pays ~6us of per-descriptor HBM write-acks)
            psF = ppool.tile([1, 2], f32)
            nc.tensor.matmul(psF[:], sm[:, 10:11], g[:], start=True, stop=True)
            outs = mpool.tile([1, 2], f32)
            nc.vector.tensor_copy(outs[:], psF[:])
            nc.sync.dma_start(out[:], outs[:])

    nc.compile()
    return nc


def get_nc():
    if "nc" not in _NC_CACHE:
        _NC_CACHE["nc"] = _build_nc()
    return _NC_CACHE["nc"]


def make_in_maps(y_pred, u_all, u_pos, index_s, n_pos):
    import ml_dtypes

    y = np.ascontiguousarray(np.asarray(y_pred, dtype=np.float32).reshape(N))
    u_all = np.asarray(u_all, dtype=np.float32).reshape(-1)
    u_pos = np.asarray(u_pos, dtype=np.float32).reshape(-1)
    idx = np.asarray(index_s).astype(np.int64).reshape(-1)[:P]
    f = y[:P]

    # sharding permutation: rows sorted by f so each core gets a band
    perm = np.argsort(f, kind="stable")
    th_s = (f[perm] - 1.0).astype(np.float32)
    ua_s = u_all[idx[perm]]
    up_s = u_pos[idx[perm]]

    # (k, jsub)-interleaved y broadcast: row p = 32k+jsub holds y[32j+jsub]
    y4 = y.reshape(FREEW, JSUB).T.astype(ml_dtypes.bfloat16)  # [JSUB, FREEW]
    yb = np.ascontiguousarray(np.tile(y4, (K, 1)))  # [128, FREEW]

    bones4 = np.zeros((128, K), dtype=np.float32)
    for k in range(K):
        bones4[JSUB * k : JSUB * (k + 1), k] = 1.0

    in_maps = []
    for c in range(NCORES):
        rows = slice(c * RPC, (c + 1) * RPC)
        th = th_s[rows]
        lo = float(th.min())
        hi = float(th.max())
        delta = max(hi - lo, 1e-3) / K
        tg = lo + delta * (np.arange(K, dtype=np.float64) + 0.5)  # bin centers
        tgb = tg.astype(np.float32).astype(ml_dtypes.bfloat16)  # snapped
        tgf = tgb.astype(np.float32)
        b = np.clip(((th - lo) / delta).astype(np.int64), 0, K - 1)
        eps = (th - tgf[b]).astype(np.float32)

        sm = np.zeros((128, 20), dtype=np.float32)
        # U: col 2h+j = {ua, up} of row h*128 + rloc (raw, un-scaled)
        sm[:, 0] = ua_s[rows][0:128]
        sm[:, 1] = up_s[rows][0:128]
        sm[:, 2] = ua_s[rows][128:256]
        sm[:, 3] = up_s[rows][128:256]
        sm[:, 4:8] = bones4
        sm[:, 8] = np.repeat(tgf, JSUB)
        sm[:, 9] = -np.repeat(tgf, JSUB)
        sm[:, 10] = 1.0

        # eps-dup weights: col 12+4h+2g+j, g=0 -> -2*eps, g=1 -> eps^2
        n2e = -2.0 * eps
        ep2 = eps * eps
        for h in range(2):
            for j in range(2):
                sm[:, 12 + 4 * h + j] = n2e[128 * h : 128 * (h + 1)]
                sm[:, 14 + 4 * h + j] = ep2[128 * h : 128 * (h + 1)]

        # plain one-hot (exact in bf16)
        oh = (b[None, :] == np.arange(K)[:, None])
        ohb = np.ascontiguousarray(oh.astype(ml_dtypes.bfloat16))

        in_maps.append(
            {
                "yb": yb.reshape(1, -1),
                "sm": np.ascontiguousarray(sm).reshape(1, -1),
                "ohb": ohb,
            }
        )
    return in_maps


def kernel(**inputs):
    n_pos = int(np.asarray(inputs["n_pos"]))
    assert n_pos == P, f"kernel hardcodes n_pos={P}, got {n_pos}"
    in_maps = make_in_maps(
        inputs["y_pred"], inputs["u_all"], inputs["u_pos"], inputs["index_s"], n_pos
    )
    from concourse.bass_utils import run_bass_kernel_spmd

    nc = get_nc()
    res = run_bass_kernel_spmd(nc, in_maps, list(range(NCORES)))
    total = 0.0
    for r in res.results:
        total += float(np.asarray(r["out"], dtype=np.float64).sum())
    return np.float32(total)


# revision 19
# speedup vs baseline: 1.5500x; 1.0033x over previous
"""Trainium2 Bass kernel for nn_APLoss (8 NeuronCores, SPMD row-sharded).

Every per-row quantity of the reference collapses to a 1-D function of
the row threshold th_i = f_i - 1:

  R(th)    = sum_j relu(y_j - th)^2        (over all 16384 j)
  Rpos(th) = same over the first 2048 j

Rows (positives) are sorted by f on the host (a sharding permutation --
the loss is a mean over rows, so permutation-invariant); core c gets a
contiguous band of 256 thresholds.  Each core evaluates the moment
functions  M(t) = sum relu(y-t),  Q(t) = sum relu(y-t)^2,
C(t) = #{y > t}  at K=4 bin centers spanning its band; per row the
EXACT 2nd-order Taylor form
  R(th) = Q(t_b) - 2*eps*M(t_b) + eps^2*C(t_b),   eps = th - t_b
(nearest bin center b) leaves only the tiny crossing residual
O(rho*eps^3) -- ~2e-5 relative, far inside the 2e-2 gate.

Device layout: partitions = (4 grid-k x 32 j-sublane), free = j-block
(512 = 16384/32).  ScalarE computes M/Q via Relu/Square activations
with free-axis accum_out (the ACT table load fires at queue-reach
time, hidden under the input DMAs); DVE computes the count pass and
the pos pieces with plain tensor_scalar + tensor_reduce.  One bones
matmul folds the 32 sublanes into A6[4,6] = k x ({M,Q,C} x {tot,pos});
one bf16 plain-one-hot matmul per 128-row half gathers each row's bin
values, and the eps-weighted Taylor combine + u-update / p / mean
chain runs on DVE in [128,2,2] row layout.  A final ones-matmul folds
the partition axis so each core stores a single [1,2] partial (a
[128,1] store costs ~6us of per-descriptor HBM write-acks); the host
sums 8 partials.

NOTE for future sessions: vector memset / iota and DVE accum_out on
tensor_scalar / scalar_tensor_tensor / tensor_tensor_reduce HANG on
this hardware path (custom-ucode ops; CoreSim and walrus accept them).
Stick to baseline-proven ops: plain TS/STT/TT, tensor_reduce,
tensor_copy, reciprocal, matmul, ScalarE activation(+accum_out).

Host-side prep is layout/sharding only: dtype casts, the row sort
permutation, per-core grid metadata (bin centers, eps, one-hot), the
gather u[index_s[rows]], and the (k,jsub)-interleaved y broadcast.
"""

import numpy as np

try:
    import concourse.bass as bass  # noqa: F401
except ImportError:  # pragma: no cover
    import sys

    sys.path.insert(0, "/opt/trn_rl_repo")

N = 16384
P = 2048
NCORES = 8
RPC = P // NCORES  # 256 rows per core
K = 4  # grid points per core
JSUB = 32  # j sublanes per partition group (K * JSUB = 128)
FREEW = N // JSUB  # 512 free columns
POSW = P // JSUB  # 64 pos free columns
GAMMA = 0.99
CP = GAMMA / ((1.0 - GAMMA) * N)  # u-update scale on R (raw-u form)
CG = 1.0 / ((1.0 - GAMMA) * float(N) * float(P))  # final scale

_NC_CACHE = {}


def _build_nc():
    import concourse.tile as tile
    from concourse import bacc, mybir

    f32 = mybir.dt.float32
    bf16 = mybir.dt.bfloat16
    Alu = mybir.AluOpType
    Act = mybir.ActivationFunctionType

    nc = bacc.Bacc("TRN2", target_bir_lowering=False, debug=False, num_devices=NCORES)

    yb_d = nc.dram_tensor("yb", [128, FREEW], bf16, kind="ExternalInput").ap()
    sm_d = nc.dram_tensor("sm", [128, 20], f32, kind="ExternalInput").ap()
    ohb_d = nc.dram_tensor("ohb", [K, 256], bf16, kind="ExternalInput").ap()
    out = nc.dram_tensor("out", [1, 2], f32, kind="ExternalOutput").ap()

    with tile.TileContext(nc) as tc:
        with (
            tc.tile_pool(name="const", bufs=1) as cpool,
            tc.tile_pool(name="work", bufs=2) as wpool,
            tc.tile_pool(name="small", bufs=1) as mpool,
            tc.tile_pool(name="psum", bufs=1, space="PSUM") as ppool,
        ):
            # --- input DMAs: yb + ohb on sync, sm on scalar (both HWDGE) ---
            Yb = cpool.tile([128, FREEW], bf16)
            nc.sync.dma_start(Yb[:], yb_d[:])
            ohb = cpool.tile([K, 256], bf16)
            nc.sync.dma_start(ohb[:], ohb_d[:])
            sm = cpool.tile([128, 20], f32)
            nc.scalar.dma_start(sm[:], sm_d[:])

            # sm columns: 0-3 U (h x {ua, up}), 4-7 bones4, 8 t128, 9 -t128,
            # 10 ones, 12-15 -2*eps dup, 16-19 eps^2 dup
            bones4 = sm[:, 4:8]
            t128 = sm[:, 8:9]
            negt128 = sm[:, 9:10]

            # --- moment pass: ScalarE Relu/Square accum for full M/Q (the
            # ACT table load runs at queue-reach, before yb arrives); DVE
            # does the count pass + pos pieces with plain ops + reduces.
            # acc6 cols: [Mt, Mp, Ct, Cp, Qt, Qp]
            acc6 = mpool.tile([128, 6], f32)
            Tf = wpool.tile([128, FREEW], f32, tag="tf")
            nc.scalar.activation(
                Tf[:], Yb[:], Act.Relu, bias=negt128, scale=1.0,
                accum_out=acc6[:, 0:1],
            )
            T2f = wpool.tile([128, FREEW], f32, tag="t2f")
            nc.scalar.activation(
                T2f[:], Tf[:], Act.Square, accum_out=acc6[:, 4:5],
            )
            # count pass on DVE straight from Yb (independent of ScalarE)
            Cmf = wpool.tile([128, FREEW], bf16, tag="cmf")
            nc.vector.tensor_scalar(
                Cmf[:], Yb[:], t128, 0.0, Alu.subtract, Alu.is_gt
            )
            nc.vector.tensor_reduce(
                acc6[:, 2:3], Cmf[:], mybir.AxisListType.X, Alu.add
            )
            # pos pieces on DVE (cols [0, POSW))
            Tp = wpool.tile([128, POSW], bf16, tag="tp")
            nc.vector.tensor_scalar(
                Tp[:], Yb[:, 0:POSW], t128, 0.0, Alu.subtract, Alu.max
            )
            nc.vector.tensor_reduce(
                acc6[:, 1:2], Tp[:], mybir.AxisListType.X, Alu.add
            )
            T2p = wpool.tile([128, POSW], bf16, tag="t2p")
            nc.vector.scalar_tensor_tensor(
                T2p[:], Tp[:], 0.0, Tp[:], Alu.add, Alu.mult
            )
            nc.vector.tensor_reduce(
                acc6[:, 5:6], T2p[:], mybir.AxisListType.X, Alu.add
            )
            nc.vector.tensor_reduce(
                acc6[:, 3:4], Cmf[:, 0:POSW], mybir.AxisListType.X, Alu.add
            )

            # --- fold 32 sublanes per grid k: A6[k, (m x {tot,pos})] ---
            psA6 = ppool.tile([K, 6], f32)
            nc.tensor.matmul(psA6[:], bones4, acc6[:], start=True, stop=True)
            A6b = mpool.tile([K, 6], bf16)
            nc.vector.tensor_copy(A6b[:], psA6[:])

            # --- per-row gather: one bf16 plain-one-hot matmul per half ---
            psV = ppool.tile([128, 2, 6], f32)
            for h in range(2):
                nc.tensor.matmul(
                    psV[:, h, :], ohb[:, h * 128 : (h + 1) * 128], A6b[:],
                    start=True, stop=True,
                )
            V = mpool.tile([128, 2, 6], f32)
            nc.vector.tensor_copy(
                V[:].rearrange("p a b -> p (a b)"),
                psV[:].rearrange("p a b -> p (a b)"),
            )

            # R = Q - 2*eps*M + eps^2*C on DVE; the {-2eps*M, eps^2*C}
            # products run as ONE op over a [128,2,2,2] AP (sm cols 12-19
            # hold the weights in matching (h, g, j) order)
            ab = mpool.tile([128, 2, 2, 2], f32)
            nc.vector.tensor_tensor(
                ab[:],
                sm[:, 12:20].rearrange("p (a g b) -> p a g b", a=2, g=2),
                V[:, :, 0:4].rearrange("p a (g b) -> p a g b", g=2),
                Alu.mult,
            )
            cv = mpool.tile([128, 2, 2], f32)
            nc.vector.tensor_tensor(cv[:], ab[:, :, 0, :], ab[:, :, 1, :], Alu.add)
            RRp = mpool.tile([128, 2, 2], f32)
            nc.vector.tensor_tensor(RRp[:], cv[:], V[:, :, 4:6], Alu.add)

            # --- u-update / p / mean (raw-u scaling folded into CP/CG) ---
            UN = mpool.tile([128, 2, 2], f32)
            nc.vector.scalar_tensor_tensor(
                UN[:].rearrange("p a b -> p (a b)"),
                RRp[:].rearrange("p a b -> p (a b)"),
                CP,
                sm[:, 0:4],
                Alu.mult, Alu.add,
            )
            w = mpool.tile([128, 2], f32)
            nc.vector.reciprocal(w[:], UN[:, :, 0])
            t1 = mpool.tile([128, 2], f32)
            nc.vector.tensor_tensor(t1[:], UN[:, :, 1], RRp[:, :, 0], Alu.mult)
            t2 = mpool.tile([128, 2], f32)
            nc.vector.tensor_tensor(t2[:], t1[:], w[:], Alu.mult)
            t3 = mpool.tile([128, 2], f32)
            nc.vector.tensor_tensor(t3[:], t2[:], RRp[:, :, 1], Alu.subtract)
            g = mpool.tile([128, 2], f32)
            nc.vector.scalar_tensor_tensor(
                g[:], t3[:], CG, w[:], Alu.mult, Alu.mult,
            )
            # fold partitions on PE so the out DMA is one 8-byte descriptor
            # (a [128,1] store pays ~6us of per-descriptor HBM write-acks)
            psF = ppool.tile([1, 2], f32)
            nc.tensor.matmul(psF[:], sm[:, 10:11], g[:], start=True, stop=True)
            outs = mpool.tile([1, 2], f32)
            nc.vector.tensor_copy(outs[:], psF[:])
            nc.sync.dma_start(out[:], outs[:])

    nc.compile()
    return nc


def get_nc():
    if "nc" not in _NC_CACHE:
        _NC_CACHE["nc"] = _build_nc()
    return _NC_CACHE["nc"]


def make_in_maps(y_pred, u_all, u_pos, index_s, n_pos):
    import ml_dtypes

    y = np.ascontiguousarray(np.asarray(y_pred, dtype=np.float32).reshape(N))
    u_all = np.asarray(u_all, dtype=np.float32).reshape(-1)
    u_pos = np.asarray(u_pos, dtype=np.float32).reshape(-1)
    idx = np.asarray(index_s).astype(np.int64).reshape(-1)[:P]
    f = y[:P]

    # sharding permutation: rows sorted by f so each core gets a band
    perm = np.argsort(f, kind="stable")
    th_s = (f[perm] - 1.0).astype(np.float32)
    ua_s = u_all[idx[perm]]
    up_s = u_pos[idx[perm]]

    # (k, jsub)-interleaved y broadcast: row p = 32k+jsub holds y[32j+jsub]
    y4 = y.reshape(FREEW, JSUB).T.astype(ml_dtypes.bfloat16)  # [JSUB, FREEW]
    yb = np.ascontiguousarray(np.tile(y4, (K, 1)))  # [128, FREEW]

    bones4 = np.zeros((128, K), dtype=np.float32)
    for k in range(K):
        bones4[JSUB * k : JSUB * (k + 1), k] = 1.0

    in_maps = []
    for c in range(NCORES):
        rows = slice(c * RPC, (c + 1) * RPC)
        th = th_s[rows]
        lo = float(th.min())
        hi = float(th.max())
        delta = max(hi - lo, 1e-3) / K
        tg = lo + delta * (np.arange(K, dtype=np.float64) + 0.5)  # bin centers
        tgb = tg.astype(np.float32).astype(ml_dtypes.bfloat16)  # snapped
        tgf = tgb.astype(np.float32)
        b = np.clip(((th - lo) / delta).astype(np.int64), 0, K - 1)
        eps = (th - tgf[b]).astype(np.float32)

        sm = np.zeros((128, 20), dtype=np.float32)
        # U: col 2h+j = {ua, up} of row h*128 + rloc (raw, un-scaled)
        sm[:, 0] = ua_s[rows][0:128]
        sm[:, 1] = up_s[rows][0:128]
        sm[:, 2] = ua_s[rows][128:256]
        sm[:, 3] = up_s[rows][128:256]
        sm[:, 4:8] = bones4
        sm[:, 8] = np.repeat(tgf, JSUB)
        sm[:, 9] = -np.repeat(tgf, JSUB)
        sm[:, 10] = 1.0

        # eps-dup weights: col 12+4h+2g+j, g=0 -> -2*eps, g=1 -> eps^2
        n2e = -2.0 * eps
        ep2 = eps * eps
        for h in range(2):
            for j in range(2):
                sm[:, 12 + 4 * h + j] = n2e[128 * h : 128 * (h + 1)]
                sm[:, 14 + 4 * h + j] = ep2[128 * h : 128 * (h + 1)]

        # plain one-hot (exact in bf16)
        oh = (b[None, :] == np.arange(K)[:, None])
        ohb = np.ascontiguousarray(oh.astype(ml_dtypes.bfloat16))

        in_maps.append(
            {
                "yb": yb,
                "sm": np.ascontiguousarray(sm),
                "ohb": ohb,
            }
        )
    return in_maps


def kernel(**inputs):
    n_pos = int(np.asarray(inputs["n_pos"]))
    assert n_pos == P, f"kernel hardcodes n_pos={P}, got {n_pos}"
    in_maps = make_in_maps(
        inputs["y_pred"], inputs["u_all"], inputs["u_pos"], inputs["index_s"], n_pos
    )
    from concourse.bass_utils import run_bass_kernel_spmd

    nc = get_nc()
    res = run_bass_kernel_spmd(nc, in_maps, list(range(NCORES)))
    total = 0.0
    for r in res.results:
        total += float(np.asarray(r["out"], dtype=np.float64).sum())
    return np.float32(total)
